# revision 1
# baseline (speedup 1.0000x reference)
"""Trainium2 Bass kernel for nn_MultiHeadAttention (B=4, S=2048, D=1024, H=16).

Sharding: 8 cores; core c handles batch b=c//2, query-row half c%2 (1024 rows).
Each core computes K/V projections for its batch's full sequence (duplicated
across the pair of cores sharing a batch -> zero collectives), Q projection for
its own rows, all 16 heads of attention, and the output projection of its rows.

All compute is kept in "transposed" orientation (feature dim on partitions):
  qhT = Wq.T-chunks.T @ qT  -> [D, RQ]   (heads are 64-partition slices)
  khT = ...               -> [D, S]
  vh_aug = vTa.T @ WvTa    -> [S, H*65]  (per-head 64 cols + a ones column)
  scoresT[k,q] per head    -> matmul(lhsT=khT_h, rhs=qhT_h), K=64
  expT = exp(scoresT/8)    -> ACT, PSUM->SBUF
  attnT[d,q] (+denominator row from the ones column of vh_aug)
  concatT = attnT * (1/denominator) broadcast  (DRAM-bounce partition bcast)
  out = concatT-chunks.T @ WoT (+ bias via K=1 ones matmul)
so the softmax reduction runs along PSUM partitions via the matmul itself and
no on-device transposes are needed.  The host pre-transposes inputs/weights.
All matmuls run in float32r (full PE rate for moving dim >= 256).
"""
import math
from contextlib import ExitStack

import ml_dtypes
import numpy as np

import concourse.bacc as bacc
import concourse.mybir as mybir
from concourse import tile
from concourse.bass_utils import run_bass_kernel_spmd

F32 = mybir.dt.float32
F32R = mybir.dt.float32r

B, S, D, H, HD = 4, 2048, 1024, 16, 64
NCORES = 8
RQ = S // 2          # query rows per core
QC = 512             # query-row chunk (psum bank width)
HA = H * 65          # vh_aug columns: per head 64 values + 1 ones column
P = 128


def build_nc_v1(s=S, rq=RQ):
    kd = D // P              # feature chunks (contraction for projections)
    kt_n = s // P            # key-row tiles
    rt_n = s // P            # v rows tiles
    nqc = rq // QC
    L = 3                    # scores->attn software pipeline lag (in kt steps)

    nc = bacc.Bacc("TRN2", target_bir_lowering=False, num_devices=NCORES)

    qT = nc.declare_dram_parameter("qT", [D, rq], F32, isOutput=False)
    kT = nc.declare_dram_parameter("kT", [D, s], F32, isOutput=False)
    vTa = nc.declare_dram_parameter("vTa", [D + 1, s], F32, isOutput=False)
    wqT = nc.declare_dram_parameter("wqT", [D, D], F32, isOutput=False)
    wkT = nc.declare_dram_parameter("wkT", [D, D], F32, isOutput=False)
    wvTa = nc.declare_dram_parameter("wvTa", [D + 1, HA], F32, isOutput=False)
    woTa = nc.declare_dram_parameter("woTa", [D + 2, D], F32, isOutput=False)
    bq = nc.declare_dram_parameter("bq", [D, 1], F32, isOutput=False)
    bk = nc.declare_dram_parameter("bk", [D, 1], F32, isOutput=False)
    out = nc.declare_dram_parameter("out", [rq, D], F32, isOutput=True)

    def r32(ap):
        return ap.bitcast(F32R)

    with (
        nc.allow_low_precision(reason="fp32r matmul operand rounding"),
        tile.TileContext(nc) as tc,
    ):
        with (
            tc.tile_pool(name="vh", bufs=1) as p_vh,
            tc.tile_pool(name="qd", bufs=1, space="DRAM") as p_qd,
            tc.tile_pool(name="kh", bufs=1) as p_kh,
        ):
            vh = [p_vh.tile([P, HA], F32R, tag=f"vh{rt}", name=f"vh{rt}") for rt in range(rt_n)]
            qd = [p_qd.tile([P, rq], F32R, tag=f"qd{m}", name=f"qd{m}") for m in range(kd)]
            kh = [p_kh.tile([P, s], F32R, tag=f"kh{m}", name=f"kh{m}") for m in range(kd)]

            # ---------------- V projection (vh_aug = v_aug @ Wv_aug) --------
            with (
                tc.tile_pool(name="wv", bufs=1) as p_wv,
                tc.tile_pool(name="vt", bufs=4) as p_vt,
                tc.tile_pool(name="psv", bufs=2, space="PSUM") as p_psv,
            ):
                wv_t = [p_wv.tile([P, HA], F32R, tag=f"wv{k}", name=f"wv{k}") for k in range(kd)]
                for k in range(kd):
                    nc.sync.dma_start(wv_t[k][:], r32(wvTa.ap()[k * P:(k + 1) * P, :]))
                wv_last = p_wv.tile([1, HA], F32R, tag="wv8")
                nc.sync.dma_start(wv_last[:], r32(wvTa.ap()[D:D + 1, :]))

                nsplits = [(0, 512), (512, 1024), (1024, HA)]
                for rt in range(rt_n):
                    ps = p_psv.tile([P, HA], F32)
                    for k in range(kd + 1):
                        if k < kd:
                            lh = p_vt.tile([P, P], F32R, tag="vt")
                            nc.sync.dma_start(
                                lh[:], r32(vTa.ap()[k * P:(k + 1) * P, rt * P:(rt + 1) * P])
                            )
                            w = wv_t[k]
                            lhs = lh[:]
                        else:
                            lh = p_vt.tile([1, P], F32R, tag="vt8")
                            nc.sync.dma_start(
                                lh[:], r32(vTa.ap()[D:D + 1, rt * P:(rt + 1) * P])
                            )
                            w = wv_last
                            lhs = lh[:]
                        for (n0, n1) in nsplits:
                            nc.tensor.matmul(
                                ps[:, n0:n1], lhs, w[:, n0:n1],
                                start=(k == 0), stop=(k == kd),
                            )
                    nc.scalar.copy(vh[rt][:], ps[:])

            # ---------------- Q / K projections (transposed outputs) -------
            def proj_T(x_ap, w_ap, b_ap, writer, ncols, col_halves):
                """writer(m, rh_slice, psum_ap, bias_tile) stores one chunk."""
                groups = [(0, 1, 2), (3, 4, 5), (6, 7)]
                with (
                    tc.tile_pool(name="wx", bufs=4) as p_w,
                    tc.tile_pool(name="xt", bufs=3) as p_x,
                    tc.tile_pool(name="bb", bufs=1) as p_b,
                    tc.tile_pool(name="psp", bufs=4, space="PSUM") as p_ps,
                ):
                    b_t = [p_b.tile([P, 1], F32, tag=f"b{m}", name=f"b{m}") for m in range(kd)]
                    for m in range(kd):
                        nc.sync.dma_start(b_t[m][:], b_ap[m * P:(m + 1) * P, :])
                    for rh in range(col_halves):
                        cw = ncols // col_halves
                        for g in groups:
                            ps = {m: p_ps.tile([P, cw], F32, tag="psp", name=f"psp{m}") for m in g}
                            for k in range(kd):
                                xt = p_x.tile([P, cw], F32R, tag="xt")
                                nc.sync.dma_start(
                                    xt[:],
                                    r32(x_ap[k * P:(k + 1) * P, rh * cw:(rh + 1) * cw]),
                                )
                                for m in g:
                                    wt = p_w.tile([P, P], F32R, tag="wt")
                                    nc.sync.dma_start(
                                        wt[:],
                                        r32(w_ap[k * P:(k + 1) * P, m * P:(m + 1) * P]),
                                    )
                                    for n0 in range(0, cw, 512):
                                        nc.tensor.matmul(
                                            ps[m][:, n0:n0 + 512],
                                            wt[:],
                                            xt[:, n0:n0 + 512],
                                            start=(k == 0), stop=(k == kd - 1),
                                        )
                            for m in g:
                                writer(m, slice(rh * cw, (rh + 1) * cw), ps[m], b_t[m])

            with tc.tile_pool(name="qtmp", bufs=3) as p_qtmp:
                def q_writer(m, cols, ps, b_t):
                    qt = p_qtmp.tile([P, cols.stop - cols.start], F32R, tag="qtmp")
                    nc.scalar.activation(
                        qt[:], ps[:],
                        mybir.ActivationFunctionType.Identity, bias=b_t[:],
                    )
                    nc.sync.dma_start(qd[m][:, cols], qt[:])

                proj_T(qT.ap(), wqT.ap(), bq.ap(), q_writer, rq, max(1, rq // 1024))

            def k_writer(m, cols, ps, b_t):
                nc.scalar.activation(
                    kh[m][:, cols], ps[:],
                    mybir.ActivationFunctionType.Identity, bias=b_t[:],
                )

            proj_T(kT.ap(), wkT.ap(), bk.ap(), k_writer, s, max(1, s // 1024))

            # ---------------- attention + output projection -----------------
            with (
                tc.tile_pool(name="cc", bufs=1) as p_cc,
                tc.tile_pool(name="wo", bufs=2) as p_wo,
                tc.tile_pool(name="oc", bufs=1) as p_oc,
                tc.tile_pool(name="qs", bufs=2) as p_qs,
                tc.tile_pool(name="rc", bufs=3) as p_rc,
                tc.tile_pool(name="rb", bufs=3) as p_rb,
                tc.tile_pool(name="ob", bufs=3) as p_ob,
                tc.tile_pool(name="rd", bufs=4, space="DRAM") as p_rd,
            ):
                cc = [p_cc.tile([P, QC], F32R, tag=f"cc{m}", name=f"cc{m}") for m in range(kd)]
                ones_t = p_oc.tile([1, P], F32R, tag="ones")
                nc.sync.dma_start(ones_t[:], r32(woTa.ap()[D + 1:D + 2, 0:P]))
                bo_t = p_oc.tile([1, D], F32R, tag="bo")
                nc.sync.dma_start(bo_t[:], r32(woTa.ap()[D:D + 1, :]))

                for qc in range(nqc):
                    with (
                        tc.tile_pool(name="pss", bufs=L + 2, space="PSUM") as p_pss,
                        tc.tile_pool(name="psa", bufs=2, space="PSUM") as p_psa,
                        tc.tile_pool(name="ex", bufs=L + 2) as p_ex,
                    ):
                        for m in range(kd):
                          qs = p_qs.tile([P, QC], F32R, tag="qs")
                          nc.sync.dma_start(qs[:], qd[m][:, qc * QC:(qc + 1) * QC])
                          for hh in range(2):
                            h, off = 2 * m + hh, hh * HD
                            acc = p_psa.tile([HD + 1, QC], F32, tag="acc")
                            ex_t = {}
                            for step in range(kt_n + L):
                                if step < kt_n:
                                    kt = step
                                    pss = p_pss.tile([P, QC], F32, tag="pss")
                                    nc.tensor.matmul(
                                        pss[:],
                                        kh[m][off:off + HD, kt * P:(kt + 1) * P],
                                        qs[off:off + HD, :],
                                        start=True, stop=True,
                                    )
                                    ex = p_ex.tile([P, QC], F32R, tag="ex")
                                    nc.scalar.activation(
                                        ex[:], pss[:],
                                        mybir.ActivationFunctionType.Exp,
                                        scale=1.0 / math.sqrt(HD),
                                    )
                                    ex_t[kt] = ex
                                if step >= L:
                                    j = step - L
                                    nc.tensor.matmul(
                                        acc[:],
                                        vh[j][:, h * 65:h * 65 + 65],
                                        ex_t.pop(j)[:],
                                        start=(j == 0), stop=(j == kt_n - 1),
                                    )
                            # normalization: r = 1/acc[64]; bcast via DRAM
                            rc = p_rc.tile([1, QC], F32, tag="rc")
                            nc.vector.reciprocal(rc[:], acc[HD:HD + 1, :])
                            rd = p_rd.tile([1, QC], F32)
                            nc.sync.dma_start(rd[:], rc[:])
                            rb = p_rb.tile([HD, QC], F32, tag="rb")
                            nc.sync.dma_start(rb[:], rd[0:1, :].to_broadcast((HD, QC)))
                            nc.vector.tensor_tensor(
                                cc[m][off:off + HD, :], acc[0:HD, :], rb[:],
                                mybir.AluOpType.mult,
                            )

                    # ---- output projection for this query chunk ----
                    with tc.tile_pool(name="pso", bufs=1, space="PSUM") as p_pso:
                        rt4 = QC // P
                        pso = {
                            (rt, n2): p_pso.tile([P, 512], F32, tag=f"o{rt}_{n2}", name=f"o{rt}_{n2}")
                            for rt in range(rt4) for n2 in range(2)
                        }
                        for d in range(kd):
                            wo_t = p_wo.tile([P, D], F32R, tag="wo")
                            nc.sync.dma_start(
                                wo_t[:], r32(woTa.ap()[d * P:(d + 1) * P, :])
                            )
                            for rt in range(rt4):
                                for n2 in range(2):
                                    nc.tensor.matmul(
                                        pso[(rt, n2)][:],
                                        cc[d][:, rt * P:(rt + 1) * P],
                                        wo_t[:, n2 * 512:(n2 + 1) * 512],
                                        start=(d == 0), stop=False,
                                    )
                        for rt in range(rt4):
                            for n2 in range(2):
                                nc.tensor.matmul(
                                    pso[(rt, n2)][:],
                                    ones_t[0:1, :],
                                    bo_t[0:1, n2 * 512:(n2 + 1) * 512],
                                    start=False, stop=True,
                                )
                                ob = p_ob.tile([P, 512], F32, tag="ob")
                                nc.scalar.copy(ob[:], pso[(rt, n2)][:])
                                nc.sync.dma_start(
                                    out.ap()[
                                        qc * QC + rt * P:qc * QC + (rt + 1) * P,
                                        n2 * 512:(n2 + 1) * 512,
                                    ],
                                    ob[:],
                                )

    nc.compile()
    return nc


BF16 = mybir.dt.bfloat16


def build_nc(s=S, rq=RQ, kv_gather=False):
    """v6: all-bf16; resident kh/vh/qh; streamed V inputs; optional pairwise
    AllGather so each core only computes K/V projections for its own rows;
    q-proj interleaved into attention as PE filler; ACT does exp only;
    division chain runs in SBUF off the PSUM critical path."""
    kd = D // P
    kt_n = s // P
    rt_n = s // P
    sh = s // 2 if kv_gather else s      # own K/V rows per core
    rt_own = sh // P
    L = 2

    nc = bacc.Bacc("TRN2", target_bir_lowering=False, num_devices=NCORES)

    def nspans(total, w=512):
        return [(a, min(a + w, total)) for a in range(0, total, w)]

    qT = nc.declare_dram_parameter("qT", [D, rq], BF16, isOutput=False)
    kT = nc.declare_dram_parameter("kT", [D, sh], BF16, isOutput=False)
    vTa = nc.declare_dram_parameter("vTa", [D + 1, sh], BF16, isOutput=False)
    wqT = nc.declare_dram_parameter("wqT", [D, D], BF16, isOutput=False)
    wkT = nc.declare_dram_parameter("wkT", [D, D], BF16, isOutput=False)
    wvTa = nc.declare_dram_parameter("wvTa", [D + 1, HA], BF16, isOutput=False)
    woTa = nc.declare_dram_parameter("woTa", [D + 2, D], BF16, isOutput=False)
    bq = nc.declare_dram_parameter("bq", [D, 1], F32, isOutput=False)
    bk = nc.declare_dram_parameter("bk", [D, 1], F32, isOutput=False)
    out = nc.declare_dram_parameter("out", [rq, D], F32, isOutput=True)

    ctx = ExitStack()
    with (
        nc.allow_low_precision(reason="bf16 kernel"),
        tile.TileContext(nc) as tc,
        ctx,
    ):
        p_vh = ctx.enter_context(tc.tile_pool(name="vh", bufs=1))
        p_qh = ctx.enter_context(tc.tile_pool(name="qh", bufs=1))
        p_kh = ctx.enter_context(tc.tile_pool(name="kh", bufs=1))
        vh = [p_vh.tile([P, HA], BF16, tag=f"vh{rt}", name=f"vh{rt}")
              for rt in range(rt_n)]
        qh = [p_qh.tile([P, rq], BF16, tag=f"qh{m}", name=f"qh{m}")
              for m in range(kd)]
        kh = [p_kh.tile([P, s], BF16, tag=f"kh{m}", name=f"kh{m}")
              for m in range(kd)]

        # k-projection residents; loads overlap the v projection
        p_wks = ctx.enter_context(tc.tile_pool(name="wks", bufs=1))
        p_kts = ctx.enter_context(tc.tile_pool(name="kts", bufs=1))
        p_b = ctx.enter_context(tc.tile_pool(name="bb", bufs=1))
        wks = [p_wks.tile([P, D], BF16, tag=f"wks{k}", name=f"wks{k}")
               for k in range(kd)]
        kts = [p_kts.tile([P, sh], BF16, tag=f"kts{k}", name=f"kts{k}")
               for k in range(kd)]
        bk_t = [p_b.tile([P, 1], F32, tag=f"bk{m}", name=f"bk{m}")
                for m in range(kd)]

        # ---------------- V projection (streamed inputs) -------------------
        QT_RT = 4                     # rt tiles per streamed vTa quarter
        if kv_gather:
            p_cd = ctx.enter_context(tc.tile_pool(name="cdram", bufs=1, space="DRAM"))
            bv_in = p_cd.tile([sh, HA], BF16, tag="bvi", name="bvi")
            bv_out = p_cd.tile([2 * sh, HA], BF16, tag="bvo", name="bvo")
            bk_in = p_cd.tile([D, sh], BF16, tag="bki", name="bki")
            bk_out = p_cd.tile([2 * D, sh], BF16, tag="bko", name="bko")
            p_gsb = ctx.enter_context(tc.tile_pool(name="gsb", bufs=3))
        with (
            tc.tile_pool(name="wv", bufs=1) as p_wv,
            tc.tile_pool(name="vq", bufs=2) as p_vq,
            tc.tile_pool(name="psv", bufs=2, space="PSUM") as p_psv,
        ):
            wv_t = [p_wv.tile([P, HA], BF16, tag=f"wv{k}", name=f"wv{k}")
                    for k in range(kd)]
            for k in range(kd):
                nc.sync.dma_start(wv_t[k][:], wvTa.ap()[k * P:(k + 1) * P, :])
            wv_last = p_wv.tile([1, HA], BF16, tag="wv8")
            nc.sync.dma_start(wv_last[:], wvTa.ap()[D:D + 1, :])

            nsplits = [(0, 512), (512, 1024), (1024, HA)]
            for q0 in range(0, rt_own, QT_RT):
                qw_ = min(QT_RT, rt_own - q0) * P
                vq = [p_vq.tile([P, qw_], BF16, tag=f"vq{k}", name=f"vq{k}_{q0}")
                      for k in range(kd)]
                vq_l = p_vq.tile([1, qw_], BF16, tag="vq8", name=f"vq8_{q0}")
                for k in range(kd):
                    nc.sync.dma_start(
                        vq[k][:], vTa.ap()[k * P:(k + 1) * P, q0 * P:q0 * P + qw_]
                    )
                nc.sync.dma_start(
                    vq_l[:], vTa.ap()[D:D + 1, q0 * P:q0 * P + qw_]
                )
                if q0 == (QT_RT if rt_own > QT_RT else 0):
                    # queue the k-proj resident loads behind the first quarter
                    for k in range(kd):
                        nc.sync.dma_start(wks[k][:], wkT.ap()[k * P:(k + 1) * P, :])
                    for k in range(kd):
                        nc.sync.dma_start(kts[k][:], kT.ap()[k * P:(k + 1) * P, :])
                    for m in range(kd):
                        nc.sync.dma_start(bk_t[m][:], bk.ap()[m * P:(m + 1) * P, :])
                for rt in range(q0, min(q0 + QT_RT, rt_own)):
                    c0 = (rt - q0) * P
                    ps = p_psv.tile([P, HA], F32)
                    for k in range(kd + 1):
                        if k < kd:
                            w, lhs = wv_t[k], vq[k][:, c0:c0 + P]
                        else:
                            w, lhs = wv_last, vq_l[:, c0:c0 + P]
                        for (n0, n1) in nsplits:
                            nc.tensor.matmul(
                                ps[:, n0:n1], lhs, w[:, n0:n1],
                                start=(k == 0), stop=(k == kd),
                            )
                    if kv_gather:
                        vsb = p_gsb.tile([P, HA], BF16, tag="vsb")
                        nc.vector.tensor_copy(vsb[:], ps[:])
                        nc.sync.dma_start(bv_in[rt * P:(rt + 1) * P, :], vsb[:])
                    else:
                        nc.vector.tensor_copy(vh[rt][:], ps[:])
            if kv_gather:
                nc.gpsimd.collective_compute(
                    "AllGather", mybir.AluOpType.bypass,
                    replica_groups=[[2 * i, 2 * i + 1] for i in range(NCORES // 2)],
                    ins=[bv_in[:]], outs=[bv_out[:]],
                )

        # ------ attention-stretch pools (loads overlap k-proj) -------------
        if True:
            p_cc = ctx.enter_context(tc.tile_pool(name="cc", bufs=1))
            p_qw = ctx.enter_context(tc.tile_pool(name="qws", bufs=1))
            p_qx = ctx.enter_context(tc.tile_pool(name="qx", bufs=3))
            p_b2 = ctx.enter_context(tc.tile_pool(name="bb2", bufs=1))
            cc = [p_cc.tile([P, rq], BF16, tag=f"cc{m}", name=f"cc{m}")
                  for m in range(kd)]
            qws = [p_qw.tile([P, D], BF16, tag=f"qws{k}", name=f"qws{k}")
                   for k in range(kd)]
            for k in range(kd):
                nc.sync.dma_start(qws[k][:], wqT.ap()[k * P:(k + 1) * P, :])
            bq_t = [p_b2.tile([P, 1], F32, tag=f"bq{m}", name=f"bq{m}")
                    for m in range(kd)]
            for m in range(kd):
                nc.sync.dma_start(bq_t[m][:], bq.ap()[m * P:(m + 1) * P, :])

            # ---------------- K projection ---------------------------------
            with tc.tile_pool(name="psp", bufs=4, space="PSUM") as p_ps:
                ch = max(1, sh // 1024)
                cw = sh // ch
                for rh in range(ch):
                    for g in [(0, 1, 2), (3, 4, 5), (6, 7)]:
                        ps = {m: p_ps.tile([P, cw], F32, tag="psp", name=f"psp{m}")
                              for m in g}
                        for k in range(kd):
                            for m in g:
                                for n0 in range(0, cw, 512):
                                    n1 = min(n0 + 512, cw)
                                    nc.tensor.matmul(
                                        ps[m][:, n0:n1],
                                        wks[k][:, m * P:(m + 1) * P],
                                        kts[k][:, rh * cw + n0:rh * cw + n1],
                                        start=(k == 0), stop=(k == kd - 1),
                                    )
                        for m in g:
                            if kv_gather:
                                ksb = p_gsb.tile([P, cw], BF16, tag="ksb")
                                nc.vector.tensor_scalar(
                                    ksb[:], ps[m][:],
                                    bk_t[m][:], None, mybir.AluOpType.add,
                                )
                                nc.sync.dma_start(
                                    bk_in[m * P:(m + 1) * P, rh * cw:(rh + 1) * cw],
                                    ksb[:],
                                )
                            else:
                                nc.vector.tensor_scalar(
                                    kh[m][:, rh * cw:(rh + 1) * cw], ps[m][:],
                                    bk_t[m][:], None, mybir.AluOpType.add,
                                )
            if kv_gather:
                nc.gpsimd.collective_compute(
                    "AllGather", mybir.AluOpType.bypass,
                    replica_groups=[[2 * i, 2 * i + 1] for i in range(NCORES // 2)],
                    ins=[bk_in[:]], outs=[bk_out[:]],
                )
                # load gathered K/V into the resident SBUF tiles
                for rt in range(rt_n):
                    nc.sync.dma_start(vh[rt][:], bv_out[rt * P:(rt + 1) * P, :])
                for m in range(kd):
                    for r in range(2):
                        nc.sync.dma_start(
                            kh[m][:, r * sh:(r + 1) * sh],
                            bk_out[r * D + m * P:r * D + (m + 1) * P, :],
                        )

            # ---------------- attention ------------------------------------
            p_wo = ctx.enter_context(tc.tile_pool(name="wo", bufs=2))
            p_oc = ctx.enter_context(tc.tile_pool(name="oc", bufs=1))
            ones_t = p_oc.tile([1, P], BF16, tag="ones")
            nc.sync.dma_start(ones_t[:], woTa.ap()[D + 1:D + 2, 0:P])
            bo_t = p_oc.tile([1, D], BF16, tag="bo")
            nc.sync.dma_start(bo_t[:], woTa.ap()[D:D + 1, :])
            with (
                tc.tile_pool(name="ex", bufs=3) as p_ex,
                tc.tile_pool(name="asb", bufs=2) as p_asb,
                tc.tile_pool(name="rcd", bufs=1) as p_rc,
                tc.tile_pool(name="rb", bufs=2) as p_rb,
                tc.tile_pool(name="rd", bufs=4, space="DRAM") as p_rd,
                tc.tile_pool(name="pss", bufs=2, space="PSUM") as p_pss,
                tc.tile_pool(name="psa", bufs=1, space="PSUM") as p_psa,
                tc.tile_pool(name="psq", bufs=1, space="PSUM") as p_psq,
            ):
                def emit_qproj(m):
                    psq = p_psq.tile([P, rq], F32, tag="psq", name=f"psq{m}")
                    for k in range(kd):
                        qx = p_qx.tile([P, rq], BF16, tag="qx")
                        nc.sync.dma_start(qx[:], qT.ap()[k * P:(k + 1) * P, :])
                        for (n0, n1) in nspans(rq):
                            nc.tensor.matmul(
                                psq[:, n0:n1],
                                qws[k][:, m * P:(m + 1) * P],
                                qx[:, n0:n1],
                                start=(k == 0), stop=(k == kd - 1),
                            )
                    nc.vector.tensor_scalar(
                        qh[m][:], psq[:], bq_t[m][:], None, mybir.AluOpType.add,
                    )

                def emit_attention(h):
                    m, off = h // 2, (h % 2) * HD
                    acc = p_psa.tile([HD + 1, rq], F32, tag="acc")
                    ex_t = {}
                    for step in range(kt_n + L):
                        if step < kt_n:
                            kt = step
                            pss = p_pss.tile([P, rq], F32, tag="pss")
                            for (n0, n1) in nspans(rq):
                                nc.tensor.matmul(
                                    pss[:, n0:n1],
                                    kh[m][off:off + HD, kt * P:(kt + 1) * P],
                                    qh[m][off:off + HD, n0:n1],
                                    start=True, stop=True,
                                )
                            ex = p_ex.tile([P, rq], BF16, tag="ex")
                            nc.scalar.activation(
                                ex[:], pss[:],
                                mybir.ActivationFunctionType.Exp,
                                scale=1.0 / math.sqrt(HD),
                            )
                            ex_t[kt] = ex
                        if step >= L:
                            j = step - L
                            exj = ex_t.pop(j)
                            for (n0, n1) in nspans(rq):
                                nc.tensor.matmul(
                                    acc[:, n0:n1],
                                    vh[j][:, h * 65:h * 65 + 65],
                                    exj[:, n0:n1],
                                    start=(j == 0), stop=(j == kt_n - 1),
                                )
                    asb = p_asb.tile([HD, rq], F32, tag="asb")
                    nc.vector.tensor_copy(asb[:], acc[0:HD, :])
                    den = p_rc.tile([1, rq], F32, tag="den")
                    nc.vector.tensor_copy(den[:], acc[HD:HD + 1, :])
                    rc = p_rc.tile([1, rq], F32, tag="rc")
                    nc.vector.reciprocal_approx_fast(rc[:], den[:])
                    rd = p_rd.tile([1, rq], F32)
                    nc.sync.dma_start(rd[:], rc[:])
                    rb = p_rb.tile([HD, rq], F32, tag="rb")
                    nc.sync.dma_start(rb[:], rd[0:1, :].to_broadcast((HD, rq)))
                    nc.vector.tensor_tensor(
                        cc[m][off:off + HD, :], asb[:], rb[:],
                        mybir.AluOpType.mult,
                    )

                pre = 3 if kv_gather else 1
                for m in range(pre):
                    emit_qproj(m)
                for m in range(kd):
                    emit_attention(2 * m)
                    emit_attention(2 * m + 1)
                    if m + pre < kd:
                        emit_qproj(m + pre)

            # ---------------- output projection -----------------------------
            with (
                tc.tile_pool(name="ob", bufs=2) as p_ob,
                tc.tile_pool(name="pso", bufs=1, space="PSUM") as p_pso,
            ):
                OW = min(512, rq)
                rt4 = OW // P
                pso = {
                    (rt, n2): p_pso.tile([P, 512], F32, tag=f"o{rt}_{n2}",
                                         name=f"o{rt}_{n2}")
                    for rt in range(rt4) for n2 in range(2)
                }
                for half in range(rq // OW):
                    for d in range(kd):
                        wo_t = p_wo.tile([P, D], BF16, tag="wo")
                        nc.sync.dma_start(wo_t[:], woTa.ap()[d * P:(d + 1) * P, :])
                        for rt in range(rt4):
                            for n2 in range(2):
                                nc.tensor.matmul(
                                    pso[(rt, n2)][:],
                                    cc[d][:, half * OW + rt * P:half * OW + (rt + 1) * P],
                                    wo_t[:, n2 * 512:(n2 + 1) * 512],
                                    start=(d == 0), stop=False,
                                )
                    for rt in range(rt4):
                        for n2 in range(2):
                            nc.tensor.matmul(
                                pso[(rt, n2)][:],
                                ones_t[0:1, :],
                                bo_t[0:1, n2 * 512:(n2 + 1) * 512],
                                start=False, stop=True,
                            )
                            ob = p_ob.tile([P, 512], F32, tag="ob")
                            nc.vector.tensor_copy(ob[:], pso[(rt, n2)][:])
                            nc.sync.dma_start(
                                out.ap()[
                                    half * OW + rt * P:half * OW + (rt + 1) * P,
                                    n2 * 512:(n2 + 1) * 512,
                                ],
                                ob[:],
                            )

    nc.compile()
    return nc


def prep_core_inputs(q, k, v, Wq, bq, Wk, bk, Wv, bv, Wo, bo, s=S, rq=RQ, dt=np.float32, kv_gather=False):
    """Build the per-core input maps (host-side shard + transpose + augment)."""
    f = np.float32
    wqT = np.ascontiguousarray(np.asarray(Wq, f).T)
    wkT = np.ascontiguousarray(np.asarray(Wk, f).T)
    woTa = np.concatenate(
        [np.asarray(Wo, f).T, np.asarray(bo, f).reshape(1, D), np.ones((1, D), f)],
        axis=0,
    )
    woTa = np.ascontiguousarray(woTa)
    wvT = np.asarray(Wv, f).T
    wvTa = np.zeros((D + 1, HA), f)
    for h in range(H):
        wvTa[0:D, h * 65:h * 65 + HD] = wvT[:, h * HD:(h + 1) * HD]
        wvTa[D, h * 65:h * 65 + HD] = np.asarray(bv, f)[h * HD:(h + 1) * HD]
        wvTa[D, h * 65 + HD] = 1.0
    bqc = np.ascontiguousarray(np.asarray(bq, f).reshape(D, 1))
    bkc = np.ascontiguousarray(np.asarray(bk, f).reshape(D, 1))

    n_cores = (np.asarray(q).shape[0] * np.asarray(q).shape[1]) // rq
    in_maps = []
    ones_row = np.ones((1, s), f)
    for c in range(n_cores):
        b, half = divmod(c, max(1, n_cores // np.asarray(q).shape[0]))
        qT_c = np.ascontiguousarray(np.asarray(q, f)[b, half * rq:(half + 1) * rq, :].T)
        if kv_gather:
            sh = s // 2
            krows = np.asarray(k, f)[b, half * sh:(half + 1) * sh, :]
            vrows = np.asarray(v, f)[b, half * sh:(half + 1) * sh, :]
            kT_c = np.ascontiguousarray(krows.T)
            vTa_c = np.ascontiguousarray(
                np.concatenate([vrows.T, ones_row[:, :sh]], axis=0)
            )
        else:
            kT_c = np.ascontiguousarray(np.asarray(k, f)[b].T)
            vTa_c = np.ascontiguousarray(
                np.concatenate([np.asarray(v, f)[b].T, ones_row], axis=0)
            )
        in_maps.append({
            "qT": qT_c.astype(dt), "kT": kT_c.astype(dt), "vTa": vTa_c.astype(dt),
            "wqT": wqT.astype(dt), "wkT": wkT.astype(dt),
            "wvTa": wvTa.astype(dt), "woTa": woTa.astype(dt),
            "bq": bqc, "bk": bkc,
        })
    return in_maps


_NC_CACHE = {}


def run(q, k, v, Wq, bq, Wk, bk, Wv, bv, Wo, bo, trace=False, version=2):
    kv_gather = version == 2
    key = ("full", S, RQ, version)
    if key not in _NC_CACHE:
        _NC_CACHE[key] = (build_nc(S, RQ, kv_gather=kv_gather) if version == 2
                          else build_nc_v1(S, RQ))
    nc = _NC_CACHE[key]
    dt = ml_dtypes.bfloat16 if version == 2 else np.float32
    in_maps = prep_core_inputs(q, k, v, Wq, bq, Wk, bk, Wv, bv, Wo, bo, dt=dt,
                               kv_gather=kv_gather)
    res = run_bass_kernel_spmd(nc, in_maps, list(range(NCORES)), trace=trace)
    Bq, Sq, Dq = np.asarray(q).shape
    full = np.empty((Bq, Sq, Dq), np.float32)
    per_b = NCORES // Bq
    for c in range(NCORES):
        b, half = divmod(c, per_b)
        full[b, half * RQ:(half + 1) * RQ, :] = res.results[c]["out"]
    return full, res


def kernel(q, k, v, Wq, bq, Wk, bk, Wv, bv, Wo, bo):
    full, _ = run(q, k, v, Wq, bq, Wk, bk, Wv, bv, Wo, bo, trace=False)
    return full



# revision 29
# speedup vs baseline: 1.0736x; 1.0736x over previous
"""Trainium2 Bass kernel for nn_MultiHeadAttention (B=4, S=2048, D=1024, H=16).

v7 sharding: 8 cores; core c handles batch b=c//2 and head-half hh=c%2
(8 of 16 heads, full 2048-query sequence).  Projections shard perfectly
(each core projects only its 8 heads' Q/K/V feature dims over the full
sequence) with no input-side collective.  The only exchange is the attention
output (concat dims) at the end: a pairwise AllGather of cc (2 x 1MB),
pipelined under q-chunked compute; each core then applies the output
projection for its 512 output columns.

The schedule is built around the scalar engine (ACT): softmax exp is
33.5M elements/core at 1 elem/cycle/lane = ~293us, the hard floor.  Scores
are produced just-in-time ahead of exp; everything else (projections,
attn*V, output projection) weaves into tensor-engine slack:
  - scores use K=64 matmuls packed by kt-parity into PE row strips 0-63 /
    64-127 (tile_position auto-derived) so consecutive key tiles run
    concurrently in the array (~2x scores throughput).
  - vh "ones" columns (softmax denominator trick) are constants: memset,
    never computed.
  - attn*V accumulates [65, 1024] per head in PSUM (64 dims + denominator).
PSUM budget: scores pss x2 (4 banks) + acc (2) + filler/proj psum (2) = 8.
"""
import math
from contextlib import ExitStack

import ml_dtypes
import numpy as np

import concourse.bacc as bacc
import concourse.mybir as mybir
from concourse import tile
from concourse.bass_utils import run_bass_kernel_spmd

F32 = mybir.dt.float32
BF16 = mybir.dt.bfloat16

B, S, D, H, HD = 4, 2048, 1024, 16, 64
NCORES = 8
HM = H // 2            # heads per core (8)
DM = HM * HD           # my concat dims / proj width (512)
MB = DM // 128         # my 128-row proj blocks (4)
HA = HM * 65           # vh cols: per head 64 dims + 1 ones col (520)
P = 128
QC = 1024              # q chunk (psum/exp width)
NQC = S // QC          # 2
KT = S // P            # key tiles (16)
KD = D // P            # contraction chunks (8)
LAG = 2                # exp -> attn*V pipeline lag (kt steps)


DEBUG = False


def build_nc():
    nc = bacc.Bacc("TRN2", target_bir_lowering=False, num_devices=NCORES)
    dbg = {}
    if DEBUG:
        dbg["asb"] = nc.declare_dram_parameter("dbg_asb", [HD + 1, QC], F32, isOutput=True)
        dbg["rb"] = nc.declare_dram_parameter("dbg_rb", [HD, QC], F32, isOutput=True)
        dbg["kh"] = nc.declare_dram_parameter("dbg_kh", [P, S // 2], BF16, isOutput=True)
        dbg["qh"] = nc.declare_dram_parameter("dbg_qh", [P, S], BF16, isOutput=True)
        dbg["vh"] = nc.declare_dram_parameter("dbg_vh", [P, HA], BF16, isOutput=True)
        dbg["ex"] = nc.declare_dram_parameter("dbg_ex", [P, QC], BF16, isOutput=True)

    qT = nc.declare_dram_parameter("qT", [D, S], BF16, isOutput=False)
    kT = nc.declare_dram_parameter("kT", [D, S], BF16, isOutput=False)
    vT2 = nc.declare_dram_parameter("vT2", [D + 1, S], BF16, isOutput=False)
    wqT = nc.declare_dram_parameter("wqT", [D, DM], BF16, isOutput=False)
    wkT = nc.declare_dram_parameter("wkT", [D, DM], BF16, isOutput=False)
    wvT2 = nc.declare_dram_parameter("wvT2", [D + 1, DM], BF16, isOutput=False)
    woTa = nc.declare_dram_parameter("woTa", [DM + 2, D], BF16, isOutput=False)
    bq = nc.declare_dram_parameter("bq", [DM, 1], F32, isOutput=False)
    bk = nc.declare_dram_parameter("bk", [DM, 1], F32, isOutput=False)
    out = nc.declare_dram_parameter("out", [S, D], F32, isOutput=True)

    ctx = ExitStack()
    with (
        nc.allow_low_precision(reason="bf16 kernel"),
        tile.TileContext(nc) as tc,
        ctx,
    ):
        # ---------------- resident SBUF tiles ---------------------------
        p_kh = ctx.enter_context(tc.tile_pool(name="kh2", bufs=1))
        p_qh = ctx.enter_context(tc.tile_pool(name="qh2", bufs=1))
        p_vh = ctx.enter_context(tc.tile_pool(name="vh", bufs=1))
        # kh2[m2]: [128, 1024]; partitions 0-63 = head m2 K feats for even
        # kt (key cols (kt//2)*128), partitions 64-127 = odd kt.
        kh2 = [p_kh.tile([P, S // 2], BF16, tag=f"kh{m}", name=f"kh{m}")
               for m in range(HM)]
        # qh2[m2]: [128, 2048]; strip 0 = head m2 Q feats, strip 1 = dup.
        qh2 = [p_qh.tile([P, S], BF16, tag=f"qh{m}", name=f"qh{m}")
               for m in range(HM)]
        # vh[kt]: [128 keys, 520]; per head 64 V dims + ones col.
        vh = [p_vh.tile([P, HA], BF16, tag=f"vh{r}", name=f"vh{r}")
              for r in range(KT)]
        for r in range(KT):
            ones_view = vh[r][:].rearrange("p (h c) -> p h c", c=65)[:, :, 64:65]
            nc.vector.memset(ones_view, 1.0)

        # weights / staging (scoped pools entered on ctx; freed at end --
        # SBUF peak is what matters and is within budget)
        p_wv = ctx.enter_context(tc.tile_pool(name="wv2", bufs=1))
        p_wk = ctx.enter_context(tc.tile_pool(name="wkb", bufs=1))
        p_wq = ctx.enter_context(tc.tile_pool(name="wqb", bufs=1))
        p_kts = ctx.enter_context(tc.tile_pool(name="kts", bufs=1))
        p_qx = ctx.enter_context(tc.tile_pool(name="qx", bufs=4))
        p_vq = ctx.enter_context(tc.tile_pool(name="vq", bufs=12))
        p_b = ctx.enter_context(tc.tile_pool(name="bias", bufs=1))
        p_wo = ctx.enter_context(tc.tile_pool(name="wo", bufs=1))
        p_oc = ctx.enter_context(tc.tile_pool(name="oc", bufs=1))

        wv2 = [p_wv.tile([P, DM], BF16, tag=f"wv{k}", name=f"wv{k}")
               for k in range(KD)]
        wv2_l = p_wv.tile([1, DM], BF16, tag="wv8", name="wv8")
        wkb = [p_wk.tile([P, DM], BF16, tag=f"wk{k}", name=f"wk{k}")
               for k in range(KD)]
        wqb = [p_wq.tile([P, DM], BF16, tag=f"wq{k}", name=f"wq{k}")
               for k in range(KD)]
        kts = [p_kts.tile([P, S], BF16, tag=f"kt{k}", name=f"kt{k}")
               for k in range(KD)]
        bk_t = [p_b.tile([P, 1], F32, tag=f"bk{m}", name=f"bk{m}")
                for m in range(MB)]
        bq_t = [p_b.tile([P, 1], F32, tag=f"bq{m}", name=f"bq{m}")
                for m in range(MB)]
        wo = [p_wo.tile([P, D], BF16, tag=f"wo{d}", name=f"wo{d}")
              for d in range(MB)]
        ones_t = p_oc.tile([1, P], BF16, tag="ones")
        bo_t = p_oc.tile([1, D], BF16, tag="bo")

        # K-proj inputs first (first scores need kh2[0]), then Q, then V.
        for k in range(KD):
            nc.sync.dma_start(wkb[k][:], wkT.ap()[k * P:(k + 1) * P, :])
        for k in range(KD):
            nc.sync.dma_start(kts[k][:], kT.ap()[k * P:(k + 1) * P, :])
        for m in range(MB):
            nc.sync.dma_start(bk_t[m][:], bk.ap()[m * P:(m + 1) * P, :])
            nc.sync.dma_start(bq_t[m][:], bq.ap()[m * P:(m + 1) * P, :])
        for k in range(KD):
            nc.sync.dma_start(wqb[k][:], wqT.ap()[k * P:(k + 1) * P, :])
        for k in range(KD):
            nc.sync.dma_start(wv2[k][:], wvT2.ap()[k * P:(k + 1) * P, :])
        nc.sync.dma_start(wv2_l[:], wvT2.ap()[D:D + 1, :])
        for d in range(MB):
            nc.sync.dma_start(wo[d][:], woTa.ap()[d * P:(d + 1) * P, :])
        nc.sync.dma_start(ones_t[:], woTa.ap()[DM + 1:DM + 2, 0:P])
        nc.sync.dma_start(bo_t[:], woTa.ap()[DM:DM + 1, :])

        # normalized attention output (my 512 concat dims, full q) -- each
        # core later applies a PARTIAL output projection over its own dims
        # for all 1024 out cols; the host adds the two partials per batch.
        p_ccm = ctx.enter_context(tc.tile_pool(name="ccm", bufs=1))
        ccm = [p_ccm.tile([P, S], BF16, tag=f"ccm{d}", name=f"ccm{d}")
               for d in range(MB)]

        # ---------------- PSUM pools ------------------------------------
        p_pss = ctx.enter_context(tc.tile_pool(name="pss", bufs=2, space="PSUM"))
        p_acc = ctx.enter_context(tc.tile_pool(name="acc", bufs=1, space="PSUM"))
        p_fil = ctx.enter_context(tc.tile_pool(name="fil", bufs=1, space="PSUM"))

        # ---------------- transient SBUF pools --------------------------
        p_ex = ctx.enter_context(tc.tile_pool(name="ex", bufs=6))
        p_sb = ctx.enter_context(tc.tile_pool(name="sb", bufs=2))
        p_asb = ctx.enter_context(tc.tile_pool(name="asb", bufs=1))
        p_rb = ctx.enter_context(tc.tile_pool(name="rb", bufs=1))
        p_rd = ctx.enter_context(tc.tile_pool(name="rd", bufs=4, space="DRAM"))
        p_ob = ctx.enter_context(tc.tile_pool(name="ob", bufs=2))

        nsp = [(0, 512), (512, 1024)]

        # ---------------- projection emitters ---------------------------
        def emit_kproj(m, half):
            """K proj for feat block m (heads 2m, 2m+1), key half -> kh2."""
            ps = p_fil.tile([P, QC], F32, tag="fil", name=f"kp{m}_{half}")
            for k in range(KD):
                for (n0, n1) in nsp:
                    nc.tensor.matmul(
                        ps[:, n0:n1],
                        wkb[k][:, m * P:(m + 1) * P],
                        kts[k][:, half * QC + n0:half * QC + n1],
                        start=(k == 0), stop=(k == KD - 1),
                    )
            # scatter into kh2 parity strips: key col j*128 (kt = 8*half+j)
            for h in range(2):
                m2 = 2 * m + h
                for j in range(8):
                    kt_i = half * 8 + j
                    strip = (kt_i % 2) * 64
                    dst = kh2[m2][strip:strip + HD,
                                  (kt_i // 2) * P:(kt_i // 2 + 1) * P]
                    nc.vector.tensor_scalar(
                        dst, ps[h * HD:(h + 1) * HD, j * P:(j + 1) * P],
                        bk_t[m][h * HD:(h + 1) * HD, :], None,
                        mybir.AluOpType.add,
                    )

        def emit_qproj(m, half):
            """Q proj for feat block m, q half -> qh2 (both strips)."""
            ps = p_fil.tile([P, QC], F32, tag="fil", name=f"qp{m}_{half}")
            for k in range(KD):
                qx = p_qx.tile([P, QC], BF16, tag="qx")
                nc.sync.dma_start(
                    qx[:], qT.ap()[k * P:(k + 1) * P, half * QC:(half + 1) * QC])
                for (n0, n1) in nsp:
                    nc.tensor.matmul(
                        ps[:, n0:n1],
                        wqb[k][:, m * P:(m + 1) * P],
                        qx[:, n0:n1],
                        start=(k == 0), stop=(k == KD - 1),
                    )
            cols = slice(half * QC, (half + 1) * QC)
            for h in range(2):
                m2 = 2 * m + h
                src = ps[h * HD:(h + 1) * HD, :]
                bias = bq_t[m][h * HD:(h + 1) * HD, :]
                nc.vector.tensor_scalar(
                    qh2[m2][0:HD, cols], src, bias, None, mybir.AluOpType.add)
                nc.vector.tensor_scalar(
                    qh2[m2][HD:P, cols], src, bias, None, mybir.AluOpType.add)

        def emit_vproj(kt_i):
            """V proj for key tile kt_i -> vh[kt_i] (64-col blocks, stride 65)."""
            ps = p_fil.tile([P, DM], F32, tag="fil", name=f"vp{kt_i}")
            for k in range(KD + 1):
                if k < KD:
                    lh = p_vq.tile([P, P], BF16, tag="vq")
                    nc.sync.dma_start(
                        lh[:], vT2.ap()[k * P:(k + 1) * P, kt_i * P:(kt_i + 1) * P])
                    w = wv2[k]
                else:
                    lh = p_vq.tile([1, P], BF16, tag="vq8")
                    nc.sync.dma_start(
                        lh[:], vT2.ap()[D:D + 1, kt_i * P:(kt_i + 1) * P])
                    w = wv2_l
                nc.tensor.matmul(ps[:], lh[:], w[:],
                                 start=(k == 0), stop=(k == KD))
            src = ps[:].rearrange("p (h c) -> p h c", c=HD)
            dst = vh[kt_i][:].rearrange("p (h c) -> p h c", c=65)[:, :, 0:HD]
            nc.vector.tensor_copy(dst, src)

        def emit_outproj(qc, qt, pool=None, tag="fil"):
            """Partial output proj (my 4 concat-dim chunks, all 1024 out
            cols) for q tile qt of chunk qc."""
            pool = pool or p_fil
            q0 = qc * QC + qt * P
            for (n0, n1) in nsp:
                ps = pool.tile([P, 512], F32, tag=tag, name=f"op{qc}_{qt}_{n0}")
                for d in range(MB):
                    nc.tensor.matmul(
                        ps[:], ccm[d][:, q0:q0 + P], wo[d][:, n0:n1],
                        start=(d == 0), stop=False,
                    )
                nc.tensor.matmul(ps[:], ones_t[0:1, :], bo_t[0:1, n0:n1],
                                 start=False, stop=True)
                ob = p_ob.tile([P, 512], F32, tag="ob")
                nc.vector.tensor_copy(ob[:], ps[:])
                nc.sync.dma_start(out.ap()[q0:q0 + P, n0:n1], ob[:])

        # ---------------- prologue (get ACT running fast) ---------------
        emit_kproj(0, 0)
        emit_kproj(0, 1)
        emit_qproj(0, 0)
        emit_vproj(0)
        emit_vproj(1)

        # ---------------- attention -------------------------------------
        # One head, one q-chunk: 8 kt-pair steps.  Within a pair the two
        # scores matmuls land on opposite PE row strips and run
        # concurrently.  `pumps` = filler closures woven into this head's
        # window (emitted BEFORE the pair's attn*V so any vh dependency is
        # already in the tensor stream).
        def emit_attention(qc, m2, pumps=(), lag_pairs=1, pump2=False):
            pumps = list(pumps)
            npair = KT // 2
            nstep = npair + lag_pairs
            if pump2:
                sched = {i: 2 for i in range(nstep)}
            else:
                sched = {}
                for i, _ in enumerate(pumps):
                    sched[round((i + 1) * nstep / (len(pumps) + 1))] = 1
            pi = 0
            acc = p_acc.tile([HD + 1, QC], F32, tag="acc", name=f"ac{qc}_{m2}")
            ex_t = {}
            for step in range(nstep):
                for _ in range(sched.get(step, 0)):
                    if pi < len(pumps):
                        pumps[pi]()
                        pi += 1
                if step < npair:
                    for kt_i in (2 * step, 2 * step + 1):
                        strip = (kt_i % 2) * HD
                        kcols = slice((kt_i // 2) * P, (kt_i // 2 + 1) * P)
                        pss = p_pss.tile([P, QC], F32, tag="pss")
                        for (n0, n1) in nsp:
                            nc.tensor.matmul(
                                pss[:, n0:n1],
                                kh2[m2][strip:strip + HD, kcols],
                                qh2[m2][strip:strip + HD,
                                        qc * QC + n0:qc * QC + n1],
                                start=True, stop=True,
                            )
                        ex = p_ex.tile([P, QC], BF16, tag="ex")
                        nc.scalar.activation(
                            ex[:], pss[:], mybir.ActivationFunctionType.Exp,
                            scale=1.0 / math.sqrt(HD),
                        )
                        ex_t[kt_i] = ex
                        if DEBUG and qc == 0 and m2 == 0 and kt_i == 0:
                            nc.sync.dma_start(dbg["ex"].ap()[:, :], ex[:])
                if step >= lag_pairs:
                    for j in (2 * (step - lag_pairs), 2 * (step - lag_pairs) + 1):
                        exj = ex_t.pop(j)
                        for (n0, n1) in nsp:
                            nc.tensor.matmul(
                                acc[:, n0:n1],
                                vh[j][:, m2 * 65:m2 * 65 + 65],
                                exj[:, n0:n1],
                                start=(j == 0), stop=(j == KT - 1),
                            )
            while pi < len(pumps):
                pumps[pi]()
                pi += 1
            # normalize off the PSUM critical path.  NB: reciprocal (custom
            # DVE op) cannot take a partition-shifted input; copy the
            # denominator row to partition 0 first.
            asb = p_asb.tile([HD + 1, QC], F32, tag="asb")
            nc.vector.tensor_copy(asb[:], acc[:])
            den = p_sb.tile([1, QC], F32, tag="den")
            nc.vector.tensor_copy(den[:], asb[HD:HD + 1, :])
            rc = p_sb.tile([1, QC], F32, tag="rc")
            nc.vector.reciprocal_approx_fast(rc[:], den[:])
            rd = p_rd.tile([1, QC], F32)
            nc.sync.dma_start(rd[:], rc[:])
            rb = p_rb.tile([HD, QC], F32, tag="rb")
            nc.sync.dma_start(rb[:], rd[0:1, :].to_broadcast((HD, QC)))
            off = (m2 % 2) * HD
            nc.vector.tensor_tensor(
                ccm[m2 // 2][off:off + HD, qc * QC:(qc + 1) * QC],
                asb[0:HD, :], rb[:], mybir.AluOpType.mult)
            if DEBUG and qc == 0 and m2 == 0:
                nc.sync.dma_start(dbg["asb"].ap()[:, :], asb[:])
                nc.sync.dma_start(dbg["rb"].ap()[:, :], rb[:])

        def V(r):
            return lambda: emit_vproj(r)

        def KP(m, h):
            return lambda: emit_kproj(m, h)

        def QP(m, h):
            return lambda: emit_qproj(m, h)

        def OP(qc, qt):
            return lambda: emit_outproj(qc, qt)

        # filler assignment: block m's K proj + qc0 Q-proj half must land
        # before head 2m; q-half-1 projections only matter for qc1.
        pump_qc0 = {
            0: [V(r) for r in range(2, 16)],
            1: [KP(1, 0), KP(1, 1), QP(1, 0)],
            2: [QP(0, 1)],
            3: [KP(2, 0), KP(2, 1), QP(2, 0)],
            4: [QP(1, 1)],
            5: [KP(3, 0), KP(3, 1), QP(3, 0)],
            6: [QP(2, 1)],
            7: [QP(3, 1)],
        }
        pump_qc1 = {
            2: [OP(0, 0), OP(0, 1)],
            3: [OP(0, 2), OP(0, 3)],
            4: [OP(0, 4)],
            5: [OP(0, 5)],
            6: [OP(0, 6)],
            7: [OP(0, 7)],
        }
        for qc in range(NQC):
            table = pump_qc0 if qc == 0 else pump_qc1
            for m2 in range(HM):
                head0 = (qc == 0 and m2 == 0)
                emit_attention(qc, m2, pumps=table.get(m2, ()),
                               lag_pairs=2 if head0 else 1, pump2=head0)
        # tail outproj double-buffers in the (now idle) scores psum slots
        for qt in range(QC // P):
            emit_outproj(1, qt, pool=p_pss, tag="pss")
        if DEBUG:
            nc.sync.dma_start(dbg["kh"].ap()[:, :], kh2[0][:])
            nc.sync.dma_start(dbg["qh"].ap()[:, :], qh2[0][:])
            nc.sync.dma_start(dbg["vh"].ap()[:, :], vh[0][:])

    nc.compile()
    return nc


def prep_core_inputs(q, k, v, Wq, bq, Wk, bk, Wv, bv, Wo, bo):
    f = np.float32
    bf = ml_dtypes.bfloat16
    q, k, v = np.asarray(q, f), np.asarray(k, f), np.asarray(v, f)
    WqT, WkT = np.asarray(Wq, f).T, np.asarray(Wk, f).T
    WvT, WoT = np.asarray(Wv, f).T, np.asarray(Wo, f).T
    bqf, bkf = np.asarray(bq, f), np.asarray(bk, f)
    bvf, bof = np.asarray(bv, f), np.asarray(bo, f)
    ones_row = np.ones((1, S), f)

    in_maps = []
    for c in range(NCORES):
        b, hh = divmod(c, 2)
        sl = slice(hh * DM, (hh + 1) * DM)   # my feat dims / out cols
        qT_c = np.ascontiguousarray(q[b].T)
        kT_c = np.ascontiguousarray(k[b].T)
        vT2_c = np.ascontiguousarray(np.concatenate([v[b].T, ones_row], 0))
        wvT2_c = np.concatenate([WvT[:, sl], bvf[sl].reshape(1, DM)], 0)
        # partial outproj: rows = my 512 concat dims, all 1024 out cols.
        # The bias "ones" row is zeroed on odd cores so the host-side sum
        # of the two partials counts the bias exactly once.
        ones_or_zero = np.ones((1, D), f) if hh == 0 else np.zeros((1, D), f)
        woTa_c = np.concatenate(
            [WoT[sl, :], bof.reshape(1, D), ones_or_zero], 0)
        in_maps.append({
            "qT": qT_c.astype(bf), "kT": kT_c.astype(bf),
            "vT2": np.ascontiguousarray(vT2_c).astype(bf),
            "wqT": np.ascontiguousarray(WqT[:, sl]).astype(bf),
            "wkT": np.ascontiguousarray(WkT[:, sl]).astype(bf),
            "wvT2": np.ascontiguousarray(wvT2_c).astype(bf),
            "woTa": np.ascontiguousarray(woTa_c).astype(bf),
            "bq": np.ascontiguousarray(bqf[sl].reshape(DM, 1)),
            "bk": np.ascontiguousarray(bkf[sl].reshape(DM, 1)),
        })
    return in_maps


_NC_CACHE = {}


def run(q, k, v, Wq, bq, Wk, bk, Wv, bv, Wo, bo, trace=False):
    if "v7" not in _NC_CACHE:
        _NC_CACHE["v7"] = build_nc()
    nc = _NC_CACHE["v7"]
    in_maps = prep_core_inputs(q, k, v, Wq, bq, Wk, bk, Wv, bv, Wo, bo)
    res = run_bass_kernel_spmd(nc, in_maps, list(range(NCORES)), trace=trace)
    full = np.empty((B, S, D), np.float32)
    for b in range(B):
        full[b] = (np.asarray(res.results[2 * b]["out"], np.float32)
                   + np.asarray(res.results[2 * b + 1]["out"], np.float32))
    return full, res


def kernel(q, k, v, Wq, bq, Wk, bk, Wv, bv, Wo, bo):
    full, _ = run(q, k, v, Wq, bq, Wk, bk, Wv, bv, Wo, bo, trace=False)
    return full


# revision 40
# speedup vs baseline: 1.0775x; 1.0036x over previous
"""Trainium2 Bass kernel for nn_MultiHeadAttention (B=4, S=2048, D=1024, H=16).

v7 sharding: 8 cores; core c handles batch b=c//2 and head-half hh=c%2
(8 of 16 heads, full 2048-query sequence).  Projections shard perfectly
(each core projects only its 8 heads' Q/K/V feature dims over the full
sequence) with no input-side collective.  The only exchange is the attention
output (concat dims) at the end: a pairwise AllGather of cc (2 x 1MB),
pipelined under q-chunked compute; each core then applies the output
projection for its 512 output columns.

The schedule is built around the scalar engine (ACT): softmax exp is
33.5M elements/core at 1 elem/cycle/lane = ~293us, the hard floor.  Scores
are produced just-in-time ahead of exp; everything else (projections,
attn*V, output projection) weaves into tensor-engine slack:
  - scores use K=64 matmuls packed by kt-parity into PE row strips 0-63 /
    64-127 (tile_position auto-derived) so consecutive key tiles run
    concurrently in the array (~2x scores throughput).
  - vh "ones" columns (softmax denominator trick) are constants: memset,
    never computed.
  - attn*V accumulates [65, 1024] per head in PSUM (64 dims + denominator).
PSUM budget: scores pss x2 (4 banks) + acc (2) + filler/proj psum (2) = 8.
"""
import math
from contextlib import ExitStack

import ml_dtypes
import numpy as np

import concourse.bacc as bacc
import concourse.mybir as mybir
from concourse import tile
from concourse.bass_utils import run_bass_kernel_spmd

F32 = mybir.dt.float32
BF16 = mybir.dt.bfloat16

B, S, D, H, HD = 4, 2048, 1024, 16, 64
NCORES = 8
HM = H // 2            # heads per core (8)
DM = HM * HD           # my concat dims / proj width (512)
MB = DM // 128         # my 128-row proj blocks (4)
HA = HM * 65           # vh cols: per head 64 dims + 1 ones col (520)
P = 128
QC = 1024              # q chunk (psum/exp width)
NQC = S // QC          # 2
KT = S // P            # key tiles (16)
KD = D // P            # contraction chunks (8)
LAG = 2                # exp -> attn*V pipeline lag (kt steps)


DEBUG = False


def build_nc():
    nc = bacc.Bacc("TRN2", target_bir_lowering=False, num_devices=NCORES)
    dbg = {}
    if DEBUG:
        dbg["asb"] = nc.declare_dram_parameter("dbg_asb", [HD + 1, QC], F32, isOutput=True)
        dbg["rb"] = nc.declare_dram_parameter("dbg_rb", [HD, QC], F32, isOutput=True)
        dbg["kh"] = nc.declare_dram_parameter("dbg_kh", [P, S // 2], BF16, isOutput=True)
        dbg["qh"] = nc.declare_dram_parameter("dbg_qh", [P, S], BF16, isOutput=True)
        dbg["vh"] = nc.declare_dram_parameter("dbg_vh", [P, HA], BF16, isOutput=True)
        dbg["ex"] = nc.declare_dram_parameter("dbg_ex", [P, QC], BF16, isOutput=True)

    qT = nc.declare_dram_parameter("qT", [D, S], BF16, isOutput=False)
    kT = nc.declare_dram_parameter("kT", [D, S], BF16, isOutput=False)
    vT2 = nc.declare_dram_parameter("vT2", [D + 1, S], BF16, isOutput=False)
    wqT = nc.declare_dram_parameter("wqT", [D, DM], BF16, isOutput=False)
    wkT = nc.declare_dram_parameter("wkT", [D, DM], BF16, isOutput=False)
    wvT2 = nc.declare_dram_parameter("wvT2", [D + 1, DM], BF16, isOutput=False)
    woTa = nc.declare_dram_parameter("woTa", [DM + 2, D], BF16, isOutput=False)
    bq = nc.declare_dram_parameter("bq", [DM, 1], F32, isOutput=False)
    bk = nc.declare_dram_parameter("bk", [DM, 1], F32, isOutput=False)
    out = nc.declare_dram_parameter("out", [S, D], F32, isOutput=True)

    ctx = ExitStack()
    with (
        nc.allow_low_precision(reason="bf16 kernel"),
        tile.TileContext(nc) as tc,
        ctx,
    ):
        # ---------------- resident SBUF tiles ---------------------------
        p_kh = ctx.enter_context(tc.tile_pool(name="kh2", bufs=1))
        p_qh = ctx.enter_context(tc.tile_pool(name="qh2", bufs=1))
        p_vh = ctx.enter_context(tc.tile_pool(name="vh", bufs=1))
        # kh2[m2]: [128, 1024]; partitions 0-63 = head m2 K feats for even
        # kt (key cols (kt//2)*128), partitions 64-127 = odd kt.
        kh2 = [p_kh.tile([P, S // 2], BF16, tag=f"kh{m}", name=f"kh{m}")
               for m in range(HM)]
        # qh2[m2]: [128, 2048]; strip 0 = head m2 Q feats, strip 1 = dup.
        qh2 = [p_qh.tile([P, S], BF16, tag=f"qh{m}", name=f"qh{m}")
               for m in range(HM)]
        # vh[kt]: [128 keys, 520]; per head 64 V dims + ones col.
        vh = [p_vh.tile([P, HA], BF16, tag=f"vh{r}", name=f"vh{r}")
              for r in range(KT)]
        for r in range(KT):
            ones_view = vh[r][:].rearrange("p (h c) -> p h c", c=65)[:, :, 64:65]
            nc.vector.memset(ones_view, 1.0)

        # weights / staging (scoped pools entered on ctx; freed at end --
        # SBUF peak is what matters and is within budget)
        p_wv = ctx.enter_context(tc.tile_pool(name="wv2", bufs=1))
        p_wk = ctx.enter_context(tc.tile_pool(name="wkb", bufs=1))
        p_wq = ctx.enter_context(tc.tile_pool(name="wqb", bufs=1))
        p_kts = ctx.enter_context(tc.tile_pool(name="kts", bufs=1))
        p_qx = ctx.enter_context(tc.tile_pool(name="qx", bufs=4))
        p_vq = ctx.enter_context(tc.tile_pool(name="vq", bufs=12))
        p_b = ctx.enter_context(tc.tile_pool(name="bias", bufs=1))
        p_wo = ctx.enter_context(tc.tile_pool(name="wo", bufs=1))
        p_oc = ctx.enter_context(tc.tile_pool(name="oc", bufs=1))

        wv2 = [p_wv.tile([P, DM], BF16, tag=f"wv{k}", name=f"wv{k}")
               for k in range(KD)]
        wv2_l = p_wv.tile([1, DM], BF16, tag="wv8", name="wv8")
        wkb = [p_wk.tile([P, DM], BF16, tag=f"wk{k}", name=f"wk{k}")
               for k in range(KD)]
        wqb = [p_wq.tile([P, DM], BF16, tag=f"wq{k}", name=f"wq{k}")
               for k in range(KD)]
        kts = [p_kts.tile([P, S], BF16, tag=f"kt{k}", name=f"kt{k}")
               for k in range(KD)]
        bk_t = [p_b.tile([P, 1], F32, tag=f"bk{m}", name=f"bk{m}")
                for m in range(MB)]
        bq_t = [p_b.tile([P, 1], F32, tag=f"bq{m}", name=f"bq{m}")
                for m in range(MB)]
        wo = [p_wo.tile([P, D], BF16, tag=f"wo{d}", name=f"wo{d}")
              for d in range(MB)]
        ones_t = p_oc.tile([1, P], BF16, tag="ones")
        bo_t = p_oc.tile([1, D], BF16, tag="bo")

        # Ramp-ordered loads: K-proj half 0 inputs first (first scores need
        # kh2[0]), then Q-proj block 0, then the rest; wo defers to after
        # the prologue.  kts loads split by key half so kproj(0,0) starts
        # after ~3MB instead of ~8MB.
        for k in range(KD):
            nc.sync.dma_start(wkb[k][:], wkT.ap()[k * P:(k + 1) * P, :])
            nc.sync.dma_start(kts[k][:, 0:QC], kT.ap()[k * P:(k + 1) * P, 0:QC])
        for m in range(MB):
            nc.sync.dma_start(bk_t[m][:], bk.ap()[m * P:(m + 1) * P, :])
            nc.sync.dma_start(bq_t[m][:], bq.ap()[m * P:(m + 1) * P, :])
        for k in range(KD):
            nc.sync.dma_start(wqb[k][:], wqT.ap()[k * P:(k + 1) * P, :])

        def load_late_weights():
            for k in range(KD):
                nc.sync.dma_start(kts[k][:, QC:S],
                                  kT.ap()[k * P:(k + 1) * P, QC:S])
            for k in range(KD):
                nc.sync.dma_start(wv2[k][:], wvT2.ap()[k * P:(k + 1) * P, :])
            nc.sync.dma_start(wv2_l[:], wvT2.ap()[D:D + 1, :])

        def load_wo():
            for d in range(MB):
                nc.sync.dma_start(wo[d][:], woTa.ap()[d * P:(d + 1) * P, :])
            nc.sync.dma_start(ones_t[:], woTa.ap()[DM + 1:DM + 2, 0:P])
            nc.sync.dma_start(bo_t[:], woTa.ap()[DM:DM + 1, :])

        # normalized attention output (my 512 concat dims, full q) -- each
        # core later applies a PARTIAL output projection over its own dims
        # for all 1024 out cols; the host adds the two partials per batch.
        p_ccm = ctx.enter_context(tc.tile_pool(name="ccm", bufs=1))
        ccm = [p_ccm.tile([P, S], BF16, tag=f"ccm{d}", name=f"ccm{d}")
               for d in range(MB)]

        # ---------------- PSUM pools ------------------------------------
        p_pss = ctx.enter_context(tc.tile_pool(name="pss", bufs=2, space="PSUM"))
        p_acc = ctx.enter_context(tc.tile_pool(name="acc", bufs=1, space="PSUM"))
        p_fil = ctx.enter_context(tc.tile_pool(name="fil", bufs=1, space="PSUM"))

        # ---------------- transient SBUF pools --------------------------
        p_ex = ctx.enter_context(tc.tile_pool(name="ex", bufs=10))
        p_sb = ctx.enter_context(tc.tile_pool(name="sb", bufs=1))
        p_asb = ctx.enter_context(tc.tile_pool(name="asb", bufs=1))
        p_rb = ctx.enter_context(tc.tile_pool(name="rb", bufs=1))
        p_rd = ctx.enter_context(tc.tile_pool(name="rd", bufs=4, space="DRAM"))
        p_ob = ctx.enter_context(tc.tile_pool(name="ob", bufs=2))

        nsp = [(0, 512), (512, 1024)]

        # ---------------- projection emitters (generators) --------------
        # Each yields between ~1us chunks of tensor-engine work so the
        # attention loop can weave them into ACT slack at fine grain.
        def emit_kproj(m, half):
            """K proj for feat block m (heads 2m, 2m+1), key half -> kh2."""
            ps = p_fil.tile([P, QC], F32, tag="fil", name=f"kp{m}_{half}")
            for k in range(KD):
                for (n0, n1) in nsp:
                    nc.tensor.matmul(
                        ps[:, n0:n1],
                        wkb[k][:, m * P:(m + 1) * P],
                        kts[k][:, half * QC + n0:half * QC + n1],
                        start=(k == 0), stop=(k == KD - 1),
                    )
                if k % 2 == 1:
                    yield
            # scatter into kh2 parity strips: key col j*128 (kt = 8*half+j)
            for h in range(2):
                m2 = 2 * m + h
                for j in range(8):
                    kt_i = half * 8 + j
                    strip = (kt_i % 2) * 64
                    dst = kh2[m2][strip:strip + HD,
                                  (kt_i // 2) * P:(kt_i // 2 + 1) * P]
                    nc.vector.tensor_scalar(
                        dst, ps[h * HD:(h + 1) * HD, j * P:(j + 1) * P],
                        bk_t[m][h * HD:(h + 1) * HD, :], None,
                        mybir.AluOpType.add,
                    )
            yield

        def emit_qproj(m, half):
            """Q proj for feat block m, q half -> qh2 (both strips)."""
            ps = p_fil.tile([P, QC], F32, tag="fil", name=f"qp{m}_{half}")
            for k in range(KD):
                qx = p_qx.tile([P, QC], BF16, tag="qx")
                nc.sync.dma_start(
                    qx[:], qT.ap()[k * P:(k + 1) * P, half * QC:(half + 1) * QC])
                for (n0, n1) in nsp:
                    nc.tensor.matmul(
                        ps[:, n0:n1],
                        wqb[k][:, m * P:(m + 1) * P],
                        qx[:, n0:n1],
                        start=(k == 0), stop=(k == KD - 1),
                    )
                if k % 2 == 1:
                    yield
            cols = slice(half * QC, (half + 1) * QC)
            for h in range(2):
                m2 = 2 * m + h
                src = ps[h * HD:(h + 1) * HD, :]
                bias = bq_t[m][h * HD:(h + 1) * HD, :]
                nc.vector.tensor_scalar(
                    qh2[m2][0:HD, cols], src, bias, None, mybir.AluOpType.add)
                nc.vector.tensor_scalar(
                    qh2[m2][HD:P, cols], src, bias, None, mybir.AluOpType.add)
            yield

        def emit_vproj(kt_i):
            """V proj for key tile kt_i -> vh[kt_i] (64-col blocks, stride 65)."""
            ps = p_fil.tile([P, DM], F32, tag="fil", name=f"vp{kt_i}")
            for k in range(KD + 1):
                if k < KD:
                    lh = p_vq.tile([P, P], BF16, tag="vq")
                    nc.sync.dma_start(
                        lh[:], vT2.ap()[k * P:(k + 1) * P, kt_i * P:(kt_i + 1) * P])
                    w = wv2[k]
                else:
                    lh = p_vq.tile([1, P], BF16, tag="vq8")
                    nc.sync.dma_start(
                        lh[:], vT2.ap()[D:D + 1, kt_i * P:(kt_i + 1) * P])
                    w = wv2_l
                nc.tensor.matmul(ps[:], lh[:], w[:],
                                 start=(k == 0), stop=(k == KD))
                if k % 4 == 3:
                    yield
            src = ps[:].rearrange("p (h c) -> p h c", c=HD)
            dst = vh[kt_i][:].rearrange("p (h c) -> p h c", c=65)[:, :, 0:HD]
            nc.vector.tensor_copy(dst, src)
            yield

        def emit_outproj(qc, qt, pool=None, tag="fil"):
            """Partial output proj (my 4 concat-dim chunks, all 1024 out
            cols) for q tile qt of chunk qc."""
            pool = pool or p_fil
            q0 = qc * QC + qt * P
            for (n0, n1) in nsp:
                ps = pool.tile([P, 512], F32, tag=tag, name=f"op{qc}_{qt}_{n0}")
                for d in range(MB):
                    nc.tensor.matmul(
                        ps[:], ccm[d][:, q0:q0 + P], wo[d][:, n0:n1],
                        start=(d == 0), stop=False,
                    )
                nc.tensor.matmul(ps[:], ones_t[0:1, :], bo_t[0:1, n0:n1],
                                 start=False, stop=True)
                ob = p_ob.tile([P, 512], F32, tag="ob")
                nc.vector.tensor_copy(ob[:], ps[:])
                nc.sync.dma_start(out.ap()[q0:q0 + P, n0:n1], ob[:])
                yield

        def drain(gen):
            for _ in gen:
                pass

        # ---------------- prologue (get ACT running fast) ---------------
        drain(emit_kproj(0, 0))
        drain(emit_qproj(0, 0))
        load_late_weights()
        drain(emit_kproj(0, 1))
        for r in range(6):
            drain(emit_vproj(r))
        load_wo()

        # ---------------- attention -------------------------------------
        # One head, one q-chunk: 8 kt-pair steps.  Within a pair the two
        # scores matmuls land on opposite PE row strips and run
        # concurrently.  `pumps` = filler closures woven into this head's
        # window (emitted BEFORE the pair's attn*V so any vh dependency is
        # already in the tensor stream).
        def emit_attention(qc, m2, pumps=(), lag_pairs=1, per_step=None):
            """One head, one q-chunk.  `pumps` = iterable of generators
            providing ~1us filler sub-quanta; `per_step` = how many
            sub-quanta to weave per kt-pair step."""
            gens = list(pumps)

            def pump(n):
                while n > 0 and gens:
                    try:
                        next(gens[0])
                        n -= 1
                    except StopIteration:
                        gens.pop(0)

            if per_step is None:
                per_step = 1 if gens else 0
            npair = KT // 2
            nstep = npair + lag_pairs
            acc = p_acc.tile([HD + 1, QC], F32, tag="acc", name=f"ac{qc}_{m2}")
            ex_t = {}
            for step in range(nstep):
                pump(per_step)
                if step < npair:
                    pair = (2 * step, 2 * step + 1)
                    pss = {}
                    for kt_i in pair:
                        pss[kt_i] = p_pss.tile([P, QC], F32, tag="pss",
                                               name=f"pss{qc}_{m2}_{kt_i}")
                    # span-major emission: strips alternate between
                    # consecutive matmuls so the two kt run concurrently
                    for (n0, n1) in nsp:
                        for kt_i in pair:
                            strip = (kt_i % 2) * HD
                            kcols = slice((kt_i // 2) * P, (kt_i // 2 + 1) * P)
                            nc.tensor.matmul(
                                pss[kt_i][:, n0:n1],
                                kh2[m2][strip:strip + HD, kcols],
                                qh2[m2][strip:strip + HD,
                                        qc * QC + n0:qc * QC + n1],
                                start=True, stop=True,
                            )
                    for kt_i in pair:
                        ex = p_ex.tile([P, QC], BF16, tag="ex")
                        nc.scalar.activation(
                            ex[:], pss[kt_i][:],
                            mybir.ActivationFunctionType.Exp,
                            scale=1.0 / math.sqrt(HD),
                        )
                        ex_t[kt_i] = ex
                        if DEBUG and qc == 0 and m2 == 0 and kt_i == 0:
                            nc.sync.dma_start(dbg["ex"].ap()[:, :], ex[:])
                if step >= lag_pairs:
                    for j in (2 * (step - lag_pairs), 2 * (step - lag_pairs) + 1):
                        exj = ex_t.pop(j)
                        for (n0, n1) in nsp:
                            nc.tensor.matmul(
                                acc[:, n0:n1],
                                vh[j][:, m2 * 65:m2 * 65 + 65],
                                exj[:, n0:n1],
                                start=(j == 0), stop=(j == KT - 1),
                            )
            pump(10 ** 6)
            # normalize off the PSUM critical path.  NB: reciprocal (custom
            # DVE op) cannot take a partition-shifted input; copy the
            # denominator row to partition 0 first.
            asb = p_asb.tile([HD + 1, QC], F32, tag="asb")
            nc.vector.tensor_copy(asb[:], acc[:])
            den = p_sb.tile([1, QC], F32, tag="den")
            nc.vector.tensor_copy(den[:], asb[HD:HD + 1, :])
            rc = p_sb.tile([1, QC], F32, tag="rc")
            nc.vector.reciprocal_approx_fast(rc[:], den[:])
            # bounce through DRAM to broadcast 1/den across partitions;
            # issued from the vector engine so these dependent DMAs never
            # block the sync engine's prefetch stream.
            rd = p_rd.tile([1, QC], F32)
            nc.gpsimd.dma_start(rd[:], rc[:])
            rb = p_rb.tile([HD, QC], F32, tag="rb")
            nc.gpsimd.dma_start(rb[:], rd[0:1, :].to_broadcast((HD, QC)))
            off = (m2 % 2) * HD
            nc.vector.tensor_tensor(
                ccm[m2 // 2][off:off + HD, qc * QC:(qc + 1) * QC],
                asb[0:HD, :], rb[:], mybir.AluOpType.mult)
            if DEBUG and qc == 0 and m2 == 0:
                nc.sync.dma_start(dbg["asb"].ap()[:, :], asb[:])
                nc.sync.dma_start(dbg["rb"].ap()[:, :], rb[:])

        # filler assignment: block m's K proj + qc0 Q-proj half must land
        # before head 2m; q-half-1 projections only matter for qc1.
        # Values are lists of generator FACTORIES (instantiated at use).
        pump_qc0 = {
            0: [(emit_vproj, (r,)) for r in range(6, 16)],
            1: [(emit_kproj, (1, 0)), (emit_kproj, (1, 1)),
                (emit_qproj, (1, 0))],
            2: [(emit_qproj, (0, 1))],
            3: [(emit_kproj, (2, 0)), (emit_kproj, (2, 1)),
                (emit_qproj, (2, 0))],
            4: [(emit_qproj, (1, 1))],
            5: [(emit_kproj, (3, 0)), (emit_kproj, (3, 1)),
                (emit_qproj, (3, 0))],
            6: [(emit_qproj, (2, 1))],
            7: [(emit_qproj, (3, 1))],
        }
        pump_qc1 = {
            2: [(emit_outproj, (0, 0)), (emit_outproj, (0, 1))],
            3: [(emit_outproj, (0, 2)), (emit_outproj, (0, 3))],
            4: [(emit_outproj, (0, 4))],
            5: [(emit_outproj, (0, 5))],
            6: [(emit_outproj, (0, 6))],
            7: [(emit_outproj, (0, 7))],
        }
        for qc in range(NQC):
            table = pump_qc0 if qc == 0 else pump_qc1
            for m2 in range(HM):
                head0 = (qc == 0 and m2 == 0)
                gens = [f(*a) for f, a in table.get(m2, ())]
                emit_attention(qc, m2, pumps=gens,
                               lag_pairs=4 if head0 else 1,
                               per_step=3 if head0 else 2)
        # tail outproj double-buffers in the (now idle) scores psum slots
        for qt in range(QC // P):
            drain(emit_outproj(1, qt, pool=p_pss, tag="pss"))
        if DEBUG:
            nc.sync.dma_start(dbg["kh"].ap()[:, :], kh2[0][:])
            nc.sync.dma_start(dbg["qh"].ap()[:, :], qh2[0][:])
            nc.sync.dma_start(dbg["vh"].ap()[:, :], vh[0][:])

    nc.compile()
    return nc


def prep_core_inputs(q, k, v, Wq, bq, Wk, bk, Wv, bv, Wo, bo):
    f = np.float32
    bf = ml_dtypes.bfloat16
    q, k, v = np.asarray(q, f), np.asarray(k, f), np.asarray(v, f)
    WqT, WkT = np.asarray(Wq, f).T, np.asarray(Wk, f).T
    WvT, WoT = np.asarray(Wv, f).T, np.asarray(Wo, f).T
    bqf, bkf = np.asarray(bq, f), np.asarray(bk, f)
    bvf, bof = np.asarray(bv, f), np.asarray(bo, f)
    ones_row = np.ones((1, S), f)

    in_maps = []
    for c in range(NCORES):
        b, hh = divmod(c, 2)
        sl = slice(hh * DM, (hh + 1) * DM)   # my feat dims / out cols
        qT_c = np.ascontiguousarray(q[b].T)
        kT_c = np.ascontiguousarray(k[b].T)
        vT2_c = np.ascontiguousarray(np.concatenate([v[b].T, ones_row], 0))
        wvT2_c = np.concatenate([WvT[:, sl], bvf[sl].reshape(1, DM)], 0)
        # partial outproj: rows = my 512 concat dims, all 1024 out cols.
        # The bias "ones" row is zeroed on odd cores so the host-side sum
        # of the two partials counts the bias exactly once.
        ones_or_zero = np.ones((1, D), f) if hh == 0 else np.zeros((1, D), f)
        woTa_c = np.concatenate(
            [WoT[sl, :], bof.reshape(1, D), ones_or_zero], 0)
        in_maps.append({
            "qT": qT_c.astype(bf), "kT": kT_c.astype(bf),
            "vT2": np.ascontiguousarray(vT2_c).astype(bf),
            "wqT": np.ascontiguousarray(WqT[:, sl]).astype(bf),
            "wkT": np.ascontiguousarray(WkT[:, sl]).astype(bf),
            "wvT2": np.ascontiguousarray(wvT2_c).astype(bf),
            "woTa": np.ascontiguousarray(woTa_c).astype(bf),
            "bq": np.ascontiguousarray(bqf[sl].reshape(DM, 1)),
            "bk": np.ascontiguousarray(bkf[sl].reshape(DM, 1)),
        })
    return in_maps


_NC_CACHE = {}


def run(q, k, v, Wq, bq, Wk, bk, Wv, bv, Wo, bo, trace=False):
    if "v7" not in _NC_CACHE:
        _NC_CACHE["v7"] = build_nc()
    nc = _NC_CACHE["v7"]
    in_maps = prep_core_inputs(q, k, v, Wq, bq, Wk, bk, Wv, bv, Wo, bo)
    res = run_bass_kernel_spmd(nc, in_maps, list(range(NCORES)), trace=trace)
    full = np.empty((B, S, D), np.float32)
    for b in range(B):
        full[b] = (np.asarray(res.results[2 * b]["out"], np.float32)
                   + np.asarray(res.results[2 * b + 1]["out"], np.float32))
    return full, res


def kernel(q, k, v, Wq, bq, Wk, bk, Wv, bv, Wo, bo):
    full, _ = run(q, k, v, Wq, bq, Wk, bk, Wv, bv, Wo, bo, trace=False)
    return full


# revision 41
# speedup vs baseline: 1.0830x; 1.0051x over previous
"""Trainium2 Bass kernel for nn_MultiHeadAttention (B=4, S=2048, D=1024, H=16).

v7 sharding: 8 cores; core c handles batch b=c//2 and head-half hh=c%2
(8 of 16 heads, full 2048-query sequence).  Projections shard perfectly
(each core projects only its 8 heads' Q/K/V feature dims over the full
sequence) with no input-side collective.  The only exchange is the attention
output (concat dims) at the end: a pairwise AllGather of cc (2 x 1MB),
pipelined under q-chunked compute; each core then applies the output
projection for its 512 output columns.

The schedule is built around the scalar engine (ACT): softmax exp is
33.5M elements/core at 1 elem/cycle/lane = ~293us, the hard floor.  Scores
are produced just-in-time ahead of exp; everything else (projections,
attn*V, output projection) weaves into tensor-engine slack:
  - scores use K=64 matmuls packed by kt-parity into PE row strips 0-63 /
    64-127 (tile_position auto-derived) so consecutive key tiles run
    concurrently in the array (~2x scores throughput).
  - vh "ones" columns (softmax denominator trick) are constants: memset,
    never computed.
  - attn*V accumulates [65, 1024] per head in PSUM (64 dims + denominator).
PSUM budget: scores pss x2 (4 banks) + acc (2) + filler/proj psum (2) = 8.
"""
import math
from contextlib import ExitStack

import ml_dtypes
import numpy as np

import concourse.bacc as bacc
import concourse.mybir as mybir
from concourse import tile
from concourse.bass_utils import run_bass_kernel_spmd

F32 = mybir.dt.float32
BF16 = mybir.dt.bfloat16

B, S, D, H, HD = 4, 2048, 1024, 16, 64
NCORES = 8
HM = H // 2            # heads per core (8)
DM = HM * HD           # my concat dims / proj width (512)
MB = DM // 128         # my 128-row proj blocks (4)
HA = HM * 65           # vh cols: per head 64 dims + 1 ones col (520)
P = 128
QC = 1024              # q chunk (psum/exp width)
NQC = S // QC          # 2
KT = S // P            # key tiles (16)
KD = D // P            # contraction chunks (8)
LAG = 2                # exp -> attn*V pipeline lag (kt steps)


DEBUG = False


def build_nc():
    nc = bacc.Bacc("TRN2", target_bir_lowering=False, num_devices=NCORES)
    dbg = {}
    if DEBUG:
        dbg["asb"] = nc.declare_dram_parameter("dbg_asb", [HD + 1, QC], F32, isOutput=True)
        dbg["rb"] = nc.declare_dram_parameter("dbg_rb", [HD, QC], F32, isOutput=True)
        dbg["kh"] = nc.declare_dram_parameter("dbg_kh", [P, S // 2], BF16, isOutput=True)
        dbg["qh"] = nc.declare_dram_parameter("dbg_qh", [P, S], BF16, isOutput=True)
        dbg["vh"] = nc.declare_dram_parameter("dbg_vh", [P, HA], BF16, isOutput=True)
        dbg["ex"] = nc.declare_dram_parameter("dbg_ex", [P, QC], BF16, isOutput=True)

    qT = nc.declare_dram_parameter("qT", [D, S], BF16, isOutput=False)
    kT = nc.declare_dram_parameter("kT", [D, S], BF16, isOutput=False)
    vT2 = nc.declare_dram_parameter("vT2", [D + 1, S], BF16, isOutput=False)
    wqT = nc.declare_dram_parameter("wqT", [D, DM], BF16, isOutput=False)
    wkT = nc.declare_dram_parameter("wkT", [D, DM], BF16, isOutput=False)
    wvT2 = nc.declare_dram_parameter("wvT2", [D + 1, DM], BF16, isOutput=False)
    woTa = nc.declare_dram_parameter("woTa", [DM + 2, D], BF16, isOutput=False)
    bq = nc.declare_dram_parameter("bq", [DM, 1], F32, isOutput=False)
    bk = nc.declare_dram_parameter("bk", [DM, 1], F32, isOutput=False)
    out = nc.declare_dram_parameter("out", [S, D], F32, isOutput=True)

    ctx = ExitStack()
    with (
        nc.allow_low_precision(reason="bf16 kernel"),
        tile.TileContext(nc) as tc,
        ctx,
    ):
        # ---------------- resident SBUF tiles ---------------------------
        p_kh = ctx.enter_context(tc.tile_pool(name="kh2", bufs=1))
        p_qh = ctx.enter_context(tc.tile_pool(name="qh2", bufs=1))
        p_vh = ctx.enter_context(tc.tile_pool(name="vh", bufs=1))
        # kh2[m2]: [128, 1024]; partitions 0-63 = head m2 K feats for even
        # kt (key cols (kt//2)*128), partitions 64-127 = odd kt.
        kh2 = [p_kh.tile([P, S // 2], BF16, tag=f"kh{m}", name=f"kh{m}")
               for m in range(HM)]
        # qh2[m2]: [128, 2048]; strip 0 = head m2 Q feats, strip 1 = dup.
        qh2 = [p_qh.tile([P, S], BF16, tag=f"qh{m}", name=f"qh{m}")
               for m in range(HM)]
        # vh[kt]: [128 keys, 520]; per head 64 V dims + ones col.
        vh = [p_vh.tile([P, HA], BF16, tag=f"vh{r}", name=f"vh{r}")
              for r in range(KT)]
        for r in range(KT):
            ones_view = vh[r][:].rearrange("p (h c) -> p h c", c=65)[:, :, 64:65]
            nc.vector.memset(ones_view, 1.0)

        # weights / staging (scoped pools entered on ctx; freed at end --
        # SBUF peak is what matters and is within budget)
        p_wv = ctx.enter_context(tc.tile_pool(name="wv2", bufs=1))
        p_wk = ctx.enter_context(tc.tile_pool(name="wkb", bufs=1))
        p_wq = ctx.enter_context(tc.tile_pool(name="wqb", bufs=1))
        p_kts = ctx.enter_context(tc.tile_pool(name="kts", bufs=1))
        p_qx = ctx.enter_context(tc.tile_pool(name="qx", bufs=4))
        p_vq = ctx.enter_context(tc.tile_pool(name="vq", bufs=12))
        p_b = ctx.enter_context(tc.tile_pool(name="bias", bufs=1))
        p_wo = ctx.enter_context(tc.tile_pool(name="wo", bufs=1))
        p_oc = ctx.enter_context(tc.tile_pool(name="oc", bufs=1))

        wv2 = [p_wv.tile([P, DM], BF16, tag=f"wv{k}", name=f"wv{k}")
               for k in range(KD)]
        wv2_l = p_wv.tile([1, DM], BF16, tag="wv8", name="wv8")
        wkb = [p_wk.tile([P, DM], BF16, tag=f"wk{k}", name=f"wk{k}")
               for k in range(KD)]
        wqb = [p_wq.tile([P, DM], BF16, tag=f"wq{k}", name=f"wq{k}")
               for k in range(KD)]
        kts = [p_kts.tile([P, S], BF16, tag=f"kt{k}", name=f"kt{k}")
               for k in range(KD)]
        bk_t = [p_b.tile([P, 1], F32, tag=f"bk{m}", name=f"bk{m}")
                for m in range(MB)]
        bq_t = [p_b.tile([P, 1], F32, tag=f"bq{m}", name=f"bq{m}")
                for m in range(MB)]
        wo = [p_wo.tile([P, D], BF16, tag=f"wo{d}", name=f"wo{d}")
              for d in range(MB)]
        ones_t = p_oc.tile([1, P], BF16, tag="ones")
        bo_t = p_oc.tile([1, D], BF16, tag="bo")

        # Ramp-ordered loads: K-proj half 0 inputs first (first scores need
        # kh2[0]), then Q-proj block 0, then the rest; wo defers to after
        # the prologue.  kts loads split by key half so kproj(0,0) starts
        # after ~3MB instead of ~8MB.
        for k in range(KD):
            nc.sync.dma_start(wkb[k][:], wkT.ap()[k * P:(k + 1) * P, :])
            nc.sync.dma_start(kts[k][:, 0:QC], kT.ap()[k * P:(k + 1) * P, 0:QC])
        for m in range(MB):
            nc.sync.dma_start(bk_t[m][:], bk.ap()[m * P:(m + 1) * P, :])
            nc.sync.dma_start(bq_t[m][:], bq.ap()[m * P:(m + 1) * P, :])
        for k in range(KD):
            nc.sync.dma_start(wqb[k][:], wqT.ap()[k * P:(k + 1) * P, :])

        def load_late_weights():
            for k in range(KD):
                nc.sync.dma_start(kts[k][:, QC:S],
                                  kT.ap()[k * P:(k + 1) * P, QC:S])
            for k in range(KD):
                nc.sync.dma_start(wv2[k][:], wvT2.ap()[k * P:(k + 1) * P, :])
            nc.sync.dma_start(wv2_l[:], wvT2.ap()[D:D + 1, :])

        def load_wo():
            for d in range(MB):
                nc.sync.dma_start(wo[d][:], woTa.ap()[d * P:(d + 1) * P, :])
            nc.sync.dma_start(ones_t[:], woTa.ap()[DM + 1:DM + 2, 0:P])
            nc.sync.dma_start(bo_t[:], woTa.ap()[DM:DM + 1, :])

        # normalized attention output (my 512 concat dims, full q) -- each
        # core later applies a PARTIAL output projection over its own dims
        # for all 1024 out cols; the host adds the two partials per batch.
        p_ccm = ctx.enter_context(tc.tile_pool(name="ccm", bufs=1))
        ccm = [p_ccm.tile([P, S], BF16, tag=f"ccm{d}", name=f"ccm{d}")
               for d in range(MB)]

        # ---------------- PSUM pools ------------------------------------
        p_pss = ctx.enter_context(tc.tile_pool(name="pss", bufs=2, space="PSUM"))
        p_acc = ctx.enter_context(tc.tile_pool(name="acc", bufs=1, space="PSUM"))
        p_fil = ctx.enter_context(tc.tile_pool(name="fil", bufs=1, space="PSUM"))

        # ---------------- transient SBUF pools --------------------------
        p_ex = ctx.enter_context(tc.tile_pool(name="ex", bufs=10))
        p_sb = ctx.enter_context(tc.tile_pool(name="sb", bufs=1))
        p_asb = ctx.enter_context(tc.tile_pool(name="asb", bufs=1))
        p_rb = ctx.enter_context(tc.tile_pool(name="rb", bufs=1))
        p_rd = ctx.enter_context(tc.tile_pool(name="rd", bufs=4, space="DRAM"))
        p_ob = ctx.enter_context(tc.tile_pool(name="ob", bufs=2))

        nsp = [(0, 512), (512, 1024)]

        # ---------------- projection emitters (generators) --------------
        # Each yields between ~1us chunks of tensor-engine work so the
        # attention loop can weave them into ACT slack at fine grain.
        def emit_kproj(m, half):
            """K proj for feat block m (heads 2m, 2m+1), key half -> kh2."""
            ps = p_fil.tile([P, QC], F32, tag="fil", name=f"kp{m}_{half}")
            for k in range(KD):
                for (n0, n1) in nsp:
                    nc.tensor.matmul(
                        ps[:, n0:n1],
                        wkb[k][:, m * P:(m + 1) * P],
                        kts[k][:, half * QC + n0:half * QC + n1],
                        start=(k == 0), stop=(k == KD - 1),
                    )
                if k % 2 == 1:
                    yield
            # scatter into kh2 parity strips: key col j*128 (kt = 8*half+j)
            for h in range(2):
                m2 = 2 * m + h
                for j in range(8):
                    kt_i = half * 8 + j
                    strip = (kt_i % 2) * 64
                    dst = kh2[m2][strip:strip + HD,
                                  (kt_i // 2) * P:(kt_i // 2 + 1) * P]
                    nc.vector.tensor_scalar(
                        dst, ps[h * HD:(h + 1) * HD, j * P:(j + 1) * P],
                        bk_t[m][h * HD:(h + 1) * HD, :], None,
                        mybir.AluOpType.add,
                    )
            yield

        def emit_qproj(m, half):
            """Q proj for feat block m, q half -> qh2 (both strips)."""
            ps = p_fil.tile([P, QC], F32, tag="fil", name=f"qp{m}_{half}")
            for k in range(KD):
                qx = p_qx.tile([P, QC], BF16, tag="qx")
                nc.sync.dma_start(
                    qx[:], qT.ap()[k * P:(k + 1) * P, half * QC:(half + 1) * QC])
                for (n0, n1) in nsp:
                    nc.tensor.matmul(
                        ps[:, n0:n1],
                        wqb[k][:, m * P:(m + 1) * P],
                        qx[:, n0:n1],
                        start=(k == 0), stop=(k == KD - 1),
                    )
                if k % 2 == 1:
                    yield
            cols = slice(half * QC, (half + 1) * QC)
            for h in range(2):
                m2 = 2 * m + h
                src = ps[h * HD:(h + 1) * HD, :]
                bias = bq_t[m][h * HD:(h + 1) * HD, :]
                nc.vector.tensor_scalar(
                    qh2[m2][0:HD, cols], src, bias, None, mybir.AluOpType.add)
                nc.vector.tensor_scalar(
                    qh2[m2][HD:P, cols], src, bias, None, mybir.AluOpType.add)
            yield

        def emit_vproj(kt_i):
            """V proj for key tile kt_i -> vh[kt_i] (64-col blocks, stride 65)."""
            ps = p_fil.tile([P, DM], F32, tag="fil", name=f"vp{kt_i}")
            for k in range(KD + 1):
                if k < KD:
                    lh = p_vq.tile([P, P], BF16, tag="vq")
                    nc.sync.dma_start(
                        lh[:], vT2.ap()[k * P:(k + 1) * P, kt_i * P:(kt_i + 1) * P])
                    w = wv2[k]
                else:
                    lh = p_vq.tile([1, P], BF16, tag="vq8")
                    nc.sync.dma_start(
                        lh[:], vT2.ap()[D:D + 1, kt_i * P:(kt_i + 1) * P])
                    w = wv2_l
                nc.tensor.matmul(ps[:], lh[:], w[:],
                                 start=(k == 0), stop=(k == KD))
                if k % 4 == 3:
                    yield
            src = ps[:].rearrange("p (h c) -> p h c", c=HD)
            dst = vh[kt_i][:].rearrange("p (h c) -> p h c", c=65)[:, :, 0:HD]
            nc.vector.tensor_copy(dst, src)
            yield

        def emit_outproj(qc, qt, pool=None, tag="fil"):
            """Partial output proj (my 4 concat-dim chunks, all 1024 out
            cols) for q tile qt of chunk qc."""
            pool = pool or p_fil
            q0 = qc * QC + qt * P
            for (n0, n1) in nsp:
                ps = pool.tile([P, 512], F32, tag=tag, name=f"op{qc}_{qt}_{n0}")
                for d in range(MB):
                    nc.tensor.matmul(
                        ps[:], ccm[d][:, q0:q0 + P], wo[d][:, n0:n1],
                        start=(d == 0), stop=False,
                    )
                nc.tensor.matmul(ps[:], ones_t[0:1, :], bo_t[0:1, n0:n1],
                                 start=False, stop=True)
                ob = p_ob.tile([P, 512], F32, tag="ob")
                nc.vector.tensor_copy(ob[:], ps[:])
                nc.sync.dma_start(out.ap()[q0:q0 + P, n0:n1], ob[:])
                yield

        def drain(gen):
            for _ in gen:
                pass

        # ---------------- prologue (get ACT running fast) ---------------
        drain(emit_kproj(0, 0))
        drain(emit_qproj(0, 0))
        load_late_weights()
        drain(emit_kproj(0, 1))
        for r in range(6):
            drain(emit_vproj(r))
        load_wo()

        # ---------------- attention -------------------------------------
        # One head, one q-chunk: 8 kt-pair steps.  Within a pair the two
        # scores matmuls land on opposite PE row strips and run
        # concurrently.  `pumps` = filler closures woven into this head's
        # window (emitted BEFORE the pair's attn*V so any vh dependency is
        # already in the tensor stream).
        def emit_attention(qc, m2, pumps=(), lag_pairs=1, per_step=None):
            """One head, one q-chunk.  `pumps` = iterable of generators
            providing ~1us filler sub-quanta; `per_step` = how many
            sub-quanta to weave per kt-pair step."""
            gens = list(pumps)

            def pump(n):
                while n > 0 and gens:
                    try:
                        next(gens[0])
                        n -= 1
                    except StopIteration:
                        gens.pop(0)

            if per_step is None:
                per_step = 1 if gens else 0
            npair = KT // 2
            nstep = npair + lag_pairs
            acc = p_acc.tile([HD + 1, QC], F32, tag="acc", name=f"ac{qc}_{m2}")
            ex_t = {}
            for step in range(nstep):
                pump(per_step)
                if step < npair:
                    # kt-major: both spans of a kt share one LDWEIGHTS; the
                    # two kt of the pair target opposite row strips, so the
                    # second group's LDW hides under the first group's MMs
                    # and the MM streams overlap in the array.
                    for kt_i in (2 * step, 2 * step + 1):
                        strip = (kt_i % 2) * HD
                        kcols = slice((kt_i // 2) * P, (kt_i // 2 + 1) * P)
                        pss = p_pss.tile([P, QC], F32, tag="pss",
                                         name=f"pss{qc}_{m2}_{kt_i}")
                        for (n0, n1) in nsp:
                            nc.tensor.matmul(
                                pss[:, n0:n1],
                                kh2[m2][strip:strip + HD, kcols],
                                qh2[m2][strip:strip + HD,
                                        qc * QC + n0:qc * QC + n1],
                                start=True, stop=True,
                            )
                        ex = p_ex.tile([P, QC], BF16, tag="ex")
                        nc.scalar.activation(
                            ex[:], pss[:],
                            mybir.ActivationFunctionType.Exp,
                            scale=1.0 / math.sqrt(HD),
                        )
                        ex_t[kt_i] = ex
                        if DEBUG and qc == 0 and m2 == 0 and kt_i == 0:
                            nc.sync.dma_start(dbg["ex"].ap()[:, :], ex[:])
                if step >= lag_pairs:
                    for j in (2 * (step - lag_pairs), 2 * (step - lag_pairs) + 1):
                        exj = ex_t.pop(j)
                        for (n0, n1) in nsp:
                            nc.tensor.matmul(
                                acc[:, n0:n1],
                                vh[j][:, m2 * 65:m2 * 65 + 65],
                                exj[:, n0:n1],
                                start=(j == 0), stop=(j == KT - 1),
                            )
            pump(10 ** 6)
            # normalize off the PSUM critical path.  NB: reciprocal (custom
            # DVE op) cannot take a partition-shifted input; copy the
            # denominator row to partition 0 first.
            asb = p_asb.tile([HD + 1, QC], F32, tag="asb")
            nc.vector.tensor_copy(asb[:], acc[:])
            den = p_sb.tile([1, QC], F32, tag="den")
            nc.vector.tensor_copy(den[:], asb[HD:HD + 1, :])
            rc = p_sb.tile([1, QC], F32, tag="rc")
            nc.vector.reciprocal_approx_fast(rc[:], den[:])
            # bounce through DRAM to broadcast 1/den across partitions;
            # issued from the vector engine so these dependent DMAs never
            # block the sync engine's prefetch stream.
            rd = p_rd.tile([1, QC], F32)
            nc.gpsimd.dma_start(rd[:], rc[:])
            rb = p_rb.tile([HD, QC], F32, tag="rb")
            nc.gpsimd.dma_start(rb[:], rd[0:1, :].to_broadcast((HD, QC)))
            off = (m2 % 2) * HD
            nc.vector.tensor_tensor(
                ccm[m2 // 2][off:off + HD, qc * QC:(qc + 1) * QC],
                asb[0:HD, :], rb[:], mybir.AluOpType.mult)
            if DEBUG and qc == 0 and m2 == 0:
                nc.sync.dma_start(dbg["asb"].ap()[:, :], asb[:])
                nc.sync.dma_start(dbg["rb"].ap()[:, :], rb[:])

        # filler assignment: block m's K proj + qc0 Q-proj half must land
        # before head 2m; q-half-1 projections only matter for qc1.
        # Values are lists of generator FACTORIES (instantiated at use).
        pump_qc0 = {
            0: [(emit_vproj, (r,)) for r in range(6, 16)],
            1: [(emit_kproj, (1, 0)), (emit_kproj, (1, 1)),
                (emit_qproj, (1, 0))],
            2: [(emit_qproj, (0, 1))],
            3: [(emit_kproj, (2, 0)), (emit_kproj, (2, 1)),
                (emit_qproj, (2, 0))],
            4: [(emit_qproj, (1, 1))],
            5: [(emit_kproj, (3, 0)), (emit_kproj, (3, 1)),
                (emit_qproj, (3, 0))],
            6: [(emit_qproj, (2, 1))],
            7: [(emit_qproj, (3, 1))],
        }
        pump_qc1 = {
            2: [(emit_outproj, (0, 0)), (emit_outproj, (0, 1))],
            3: [(emit_outproj, (0, 2)), (emit_outproj, (0, 3))],
            4: [(emit_outproj, (0, 4))],
            5: [(emit_outproj, (0, 5))],
            6: [(emit_outproj, (0, 6))],
            7: [(emit_outproj, (0, 7))],
        }
        for qc in range(NQC):
            table = pump_qc0 if qc == 0 else pump_qc1
            for m2 in range(HM):
                head0 = (qc == 0 and m2 == 0)
                gens = [f(*a) for f, a in table.get(m2, ())]
                emit_attention(qc, m2, pumps=gens,
                               lag_pairs=4 if head0 else 1,
                               per_step=3 if head0 else 2)
        # tail outproj double-buffers in the (now idle) scores psum slots
        for qt in range(QC // P):
            drain(emit_outproj(1, qt, pool=p_pss, tag="pss"))
        if DEBUG:
            nc.sync.dma_start(dbg["kh"].ap()[:, :], kh2[0][:])
            nc.sync.dma_start(dbg["qh"].ap()[:, :], qh2[0][:])
            nc.sync.dma_start(dbg["vh"].ap()[:, :], vh[0][:])

    nc.compile()
    return nc


def prep_core_inputs(q, k, v, Wq, bq, Wk, bk, Wv, bv, Wo, bo):
    f = np.float32
    bf = ml_dtypes.bfloat16
    q, k, v = np.asarray(q, f), np.asarray(k, f), np.asarray(v, f)
    WqT, WkT = np.asarray(Wq, f).T, np.asarray(Wk, f).T
    WvT, WoT = np.asarray(Wv, f).T, np.asarray(Wo, f).T
    bqf, bkf = np.asarray(bq, f), np.asarray(bk, f)
    bvf, bof = np.asarray(bv, f), np.asarray(bo, f)
    ones_row = np.ones((1, S), f)

    in_maps = []
    for c in range(NCORES):
        b, hh = divmod(c, 2)
        sl = slice(hh * DM, (hh + 1) * DM)   # my feat dims / out cols
        qT_c = np.ascontiguousarray(q[b].T)
        kT_c = np.ascontiguousarray(k[b].T)
        vT2_c = np.ascontiguousarray(np.concatenate([v[b].T, ones_row], 0))
        wvT2_c = np.concatenate([WvT[:, sl], bvf[sl].reshape(1, DM)], 0)
        # partial outproj: rows = my 512 concat dims, all 1024 out cols.
        # The bias "ones" row is zeroed on odd cores so the host-side sum
        # of the two partials counts the bias exactly once.
        ones_or_zero = np.ones((1, D), f) if hh == 0 else np.zeros((1, D), f)
        woTa_c = np.concatenate(
            [WoT[sl, :], bof.reshape(1, D), ones_or_zero], 0)
        in_maps.append({
            "qT": qT_c.astype(bf), "kT": kT_c.astype(bf),
            "vT2": np.ascontiguousarray(vT2_c).astype(bf),
            "wqT": np.ascontiguousarray(WqT[:, sl]).astype(bf),
            "wkT": np.ascontiguousarray(WkT[:, sl]).astype(bf),
            "wvT2": np.ascontiguousarray(wvT2_c).astype(bf),
            "woTa": np.ascontiguousarray(woTa_c).astype(bf),
            "bq": np.ascontiguousarray(bqf[sl].reshape(DM, 1)),
            "bk": np.ascontiguousarray(bkf[sl].reshape(DM, 1)),
        })
    return in_maps


_NC_CACHE = {}


def run(q, k, v, Wq, bq, Wk, bk, Wv, bv, Wo, bo, trace=False):
    if "v7" not in _NC_CACHE:
        _NC_CACHE["v7"] = build_nc()
    nc = _NC_CACHE["v7"]
    in_maps = prep_core_inputs(q, k, v, Wq, bq, Wk, bk, Wv, bv, Wo, bo)
    res = run_bass_kernel_spmd(nc, in_maps, list(range(NCORES)), trace=trace)
    full = np.empty((B, S, D), np.float32)
    for b in range(B):
        full[b] = (np.asarray(res.results[2 * b]["out"], np.float32)
                   + np.asarray(res.results[2 * b + 1]["out"], np.float32))
    return full, res


def kernel(q, k, v, Wq, bq, Wk, bk, Wv, bv, Wo, bo):
    full, _ = run(q, k, v, Wq, bq, Wk, bk, Wv, bv, Wo, bo, trace=False)
    return full
